# revision 38
# baseline (speedup 1.0000x reference)
"""Block-circulant SwiGLU feed-forward (CirculantFeedForward) for 8 trn2 cores.

Sharding: token-parallel across the 8 cores (16384 tokens -> 2048/core, no
collectives).  All weights are tiny circulant generators; host-side prep
turns them into matmul tiles packed in the exact SBUF layout.

Default mode "hart3" (~790 us HW, rel err 9.1e-3, vs 1074 us for the prior
"hart" baseline): block-circulant matmuls block-diagonalize in the real
Hartley basis as 2x2 pair blocks.  Two structural tricks on top of "hart":

1. Col-tiled mixes: frequency pair-units are packed so the 4-way block sums
   (over p for gate/up, over q-groups for down) live in the contract dim,
   and unit-groups map to 32-wide PE column strips.  Four concurrent M=32
   col-tiled matmuls (tile_position) replace four serial 128x128 ones, so
   each mix stage runs ~2.3-4x denser on the PE array.
2. Radix-4 two-stage transforms for the d_ff-sized directions: the 512-pt
   DHT factors as combine @ (I_4 (x) H_128) on time-decimated samples; with
   units ordered by "quad" (the 4 frequencies sharing a sub-transform
   frequency) the combine is itself a col-tiled sparse stage.  This halves
   the dense-transform matmul volume for the iH-after-mix (C) and
   H-before-downmix (D) stages.

Per 512-token chunk the PE slot count drops 1184 ("hart") -> ~610.  The
remaining gap to the slot model is LDWEIGHTS overhead on M=32 col-tiled
matmuls (~90ns/MM vs 54ns ideal) plus copy-latency coupling in the
B->Cc->C2 pipeline (PSUM evacuation on Scalar/Vector is the co-bottleneck).
Modes "hart2" (col-tiled mixes only), "hart", "bf16", "f32r", "f32" are
kept as fallbacks.
"""
import os
from contextlib import ExitStack

import numpy as np
import ml_dtypes

import concourse.bacc as bacc
import concourse.mybir as mybir
import concourse.tile as tile
from concourse.bass_utils import run_bass_kernel_spmd

N_CORES = 8
P = 128
B = 512
D_MODEL = 2048
D_FF = 5632
KT = D_MODEL // P    # 16 k-tiles (gate/up contraction; also down output tiles)
MT = D_FF // P       # 44 m-tiles (gate/up output; down contraction)
TOK_TOTAL = 16384
TOK_CORE = TOK_TOTAL // N_CORES  # 2048

MODE = os.environ.get("BASS_MODE", "hart3")  # hart3 | hart2 | hart | bf16 | f32 | f32r

_MODE_CFG = {
    # (mybir dtype, numpy dtype, tokens per pass, matmul N, wgu bufs)
    "bf16": (mybir.dt.bfloat16, ml_dtypes.bfloat16, 1024, 512, 3),
    "f32":  (mybir.dt.float32, np.float32, 512, 512, 2),
    "f32r": (mybir.dt.float32r, np.float32, 512, 512, 2),
}
KCH = 22  # down-weight chunk: MT=44 split into 2 chunks of 22 k-tiles

_built = {}
last_results = None

# ---------------------------------------------------------------------------
# Hartley (DHT) hybrid mode: block-circulant matmul block-diagonalizes in the
# real Hartley basis with (f, B-f) pairs interleaved.  Per 512-block:
#   y_q = iH @ sum_p Mix_qp @ (H @ x_p)
# where Mix_qp is 2x2-block-diagonal => its [128,128] tiles are diagonal.
# PE work per token drops from 2112 to 1184 matmul-tiles.
# ---------------------------------------------------------------------------
HCHUNK = 512   # tokens per chunk in hart mode
N_HCH = TOK_CORE // HCHUNK


def _hartley_mats():
    f = np.arange(B)
    M = np.outer(f, f) * (2 * np.pi / B)
    H = np.cos(M) + np.sin(M)
    order = [0, B // 2] + [v for u in range(1, B // 2) for v in (u, B - u)]
    R = np.array(order)
    T_f = (H[R].T).astype(np.float32)      # [in x, out hx]
    T_i = (H[R] / B).astype(np.float32)    # [in hx, out x]
    return T_f, T_i, R


def _mix_tiles(w, R):
    """w: (q, p, B) -> mix tiles [q, p, 4, 128, 128] ([in-row, out-col])."""
    q, p, _ = w.shape
    W = np.fft.fft(w, axis=-1)
    # DHT of w: W_h[f] = Re(W[f]) - Im(W[f])  (since cas = cos + sin)
    Wh = (W.real - W.imag).astype(np.float64)   # [q, p, B]
    fs = R[0::2]                                 # +side freq of each unit
    gs = R[1::2]
    Wp = (Wh[..., fs] + Wh[..., gs]) / 2         # [q, p, 256]
    Wm = (Wh[..., fs] - Wh[..., gs]) / 2
    # unit 0 is self-paired (f=0, g=B/2): block = diag(Wh[0], Wh[B/2])
    blocks = np.zeros((q, p, B // 2, 2, 2), np.float64)
    blocks[..., 0, 0] = Wp
    blocks[..., 0, 1] = -Wm
    blocks[..., 1, 0] = Wm
    blocks[..., 1, 1] = Wp
    blocks[..., 0, 0, 0] = Wh[..., 0]
    blocks[..., 0, 0, 1] = 0.0
    blocks[..., 0, 1, 0] = 0.0
    blocks[..., 0, 1, 1] = Wh[..., B // 2]
    # assemble [q, p, kt, 128, 128]: 64 units per kt-tile
    tiles = np.zeros((q, p, 4, P, P), np.float32)
    for u_lo in range(4):
        bb = blocks[:, :, u_lo * 64:(u_lo + 1) * 64]      # [q,p,64,2,2]
        t = np.zeros((q, p, 64, 2, 64, 2), np.float64)
        iu = np.arange(64)
        t[:, :, iu, :, iu, :] = bb.transpose(2, 0, 1, 3, 4)
        tiles[:, :, u_lo] = t.reshape(q, p, P, P)
    return tiles


def _build(mode):
    if mode in _built:
        return _built[mode]
    cdt, _, pass_t, mm_n, wgu_bufs = _MODE_CFG[mode]
    n_pass = TOK_CORE // pass_t
    n_nt = pass_t // mm_n  # matmul n-tiles per pass

    nc = bacc.Bacc("TRN2", debug=False, num_devices=N_CORES)
    f32 = mybir.dt.float32

    xT = nc.dram_tensor("xT", [n_pass, P, KT * pass_t], cdt, kind="ExternalInput").ap()
    wgu = nc.dram_tensor("wgu", [MT, P, 2 * KT * P], cdt, kind="ExternalInput").ap()
    wd = nc.dram_tensor("wd", [KT, P, MT * P], cdt, kind="ExternalInput").ap()
    out = nc.dram_tensor("outT", [KT, P, TOK_CORE], f32, kind="ExternalOutput").ap()

    with tile.TileContext(nc) as tc, ExitStack() as ctx:
        xp = ctx.enter_context(tc.tile_pool(name="xp", bufs=1))
        wp = ctx.enter_context(tc.tile_pool(name="wp", bufs=wgu_bufs))
        wdp_ = ctx.enter_context(tc.tile_pool(name="wdp", bufs=2))
        hp = ctx.enter_context(tc.tile_pool(name="hp", bufs=1))
        sp = ctx.enter_context(tc.tile_pool(name="sp", bufs=2))
        op = ctx.enter_context(tc.tile_pool(name="op", bufs=2))
        ps = ctx.enter_context(tc.tile_pool(name="ps", bufs=2, space="PSUM"))

        for ip in range(n_pass):
            x_sb = xp.tile([P, KT, pass_t], cdt, tag="x")
            nc.sync.dma_start(out=x_sb, in_=xT[ip].rearrange("p (kt t) -> p kt t", kt=KT))
            h_sb = hp.tile([P, MT, pass_t], cdt, tag="h")

            # ---- gate/up + SiLU*mul ----
            for m in range(MT):
                w_sb = wp.tile([P, 2, KT, P], cdt, tag="wgu")
                nc.sync.dma_start(
                    out=w_sb, in_=wgu[m].rearrange("p (g kt q) -> p g kt q", g=2, kt=KT)
                )
                pg = ps.tile([P, pass_t], f32, tag="a")
                pu = ps.tile([P, pass_t], f32, tag="b")
                for k in range(KT):
                    for j in range(n_nt):
                        nc.tensor.matmul(
                            pg[:, j * mm_n:(j + 1) * mm_n], w_sb[:, 0, k, :],
                            x_sb[:, k, j * mm_n:(j + 1) * mm_n],
                            start=(k == 0), stop=(k == KT - 1),
                        )
                    for j in range(n_nt):
                        nc.tensor.matmul(
                            pu[:, j * mm_n:(j + 1) * mm_n], w_sb[:, 1, k, :],
                            x_sb[:, k, j * mm_n:(j + 1) * mm_n],
                            start=(k == 0), stop=(k == KT - 1),
                        )
                sg = sp.tile([P, pass_t], f32, tag="sg")
                nc.scalar.activation(sg, pg, mybir.ActivationFunctionType.Silu)
                nc.vector.tensor_mul(h_sb[:, m, :], sg, pu)

            # ---- down ----
            for m2 in range(KT):
                pd = ps.tile([P, pass_t], f32, tag="a")
                for ch in range(MT // KCH):
                    wd_sb = wdp_.tile([P, KCH, P], cdt, tag="wd")
                    nc.sync.dma_start(
                        out=wd_sb,
                        in_=wd[m2][:, ch * KCH * P:(ch + 1) * KCH * P].rearrange(
                            "p (kt q) -> p kt q", kt=KCH
                        ),
                    )
                    for kc in range(KCH):
                        k2 = ch * KCH + kc
                        for j in range(n_nt):
                            nc.tensor.matmul(
                                pd[:, j * mm_n:(j + 1) * mm_n], wd_sb[:, kc, :],
                                h_sb[:, k2, j * mm_n:(j + 1) * mm_n],
                                start=(k2 == 0), stop=(k2 == MT - 1),
                            )
                o_sb = op.tile([P, pass_t], f32, tag="o")
                nc.vector.tensor_copy(o_sb, pd)
                nc.sync.dma_start(
                    out=out[m2][:, ip * pass_t:(ip + 1) * pass_t], in_=o_sb
                )

    nc.finalize()
    _built[mode] = nc
    return nc


def _build_hart():
    if "hart" in _built:
        return _built["hart"]
    cdt = mybir.dt.bfloat16
    f32 = mybir.dt.float32
    nc = bacc.Bacc("TRN2", debug=False, num_devices=N_CORES)

    xT = nc.dram_tensor("xT", [N_HCH, P, KT * HCHUNK], cdt, kind="ExternalInput").ap()
    tf = nc.dram_tensor("tf", [P, 16 * P], cdt, kind="ExternalInput").ap()
    ti = nc.dram_tensor("ti", [P, 16 * P], cdt, kind="ExternalInput").ap()
    mgu = nc.dram_tensor("mgu", [44, P, 8 * P], cdt, kind="ExternalInput").ap()
    md = nc.dram_tensor("md", [16, P, 11 * P], cdt, kind="ExternalInput").ap()
    out = nc.dram_tensor("outT", [KT, P, TOK_CORE], f32, kind="ExternalOutput").ap()

    with tile.TileContext(nc) as tc, ExitStack() as ctx:
        xp = ctx.enter_context(tc.tile_pool(name="xp", bufs=8))
        tp = ctx.enter_context(tc.tile_pool(name="tp", bufs=1))
        mwp = ctx.enter_context(tc.tile_pool(name="mwp", bufs=4))
        ap_ = ctx.enter_context(tc.tile_pool(name="ap", bufs=24))
        sp = ctx.enter_context(tc.tile_pool(name="sp", bufs=2))
        op = ctx.enter_context(tc.tile_pool(name="op", bufs=2))
        ps = ctx.enter_context(tc.tile_pool(name="ps", bufs=2, space="PSUM"))
        ps3 = ctx.enter_context(tc.tile_pool(name="ps3", bufs=3, space="PSUM"))
        ps1 = ctx.enter_context(tc.tile_pool(name="ps1", bufs=1, space="PSUM"))

        tf_sb = tp.tile([P, 4, 4, P], cdt, tag="tf")
        ti_sb = tp.tile([P, 4, 4, P], cdt, tag="ti")
        # warm the PE (HAM un-throttle) while the first DMAs are in flight
        wz = ap_.tile([P, HCHUNK], cdt, tag="act")
        nc.vector.memset(wz, 0.0)
        wps = ps.tile([P, H2], f32, tag="tb")
        for wi in range(20):
            nc.tensor.matmul(wz_ := wps, wz[:, :P], wz, start=(wi == 0), stop=(wi == 19))
        wdrain = ap_.tile([P, 4], f32, tag="wdrain")
        nc.vector.tensor_copy(wdrain, wps[:, :4])
        tf_r = tf.rearrange("p (a b m) -> p a b m", a=4, b=4)
        for b_ in range(4):
            nc.sync.dma_start(out=tf_sb[:, :, b_], in_=tf_r[:, :, b_])
        nc.sync.dma_start(out=ti_sb, in_=ti.rearrange("p (a b m) -> p a b m", a=4, b=4))

        def load_x(ic):
            ts_ = []
            for p_ in range(4):
                t = xp.tile([P, 4, HCHUNK], cdt, tag="x")
                xr = xT[ic][:, p_ * 4 * HCHUNK:(p_ + 1) * 4 * HCHUNK].rearrange(
                    "p (kt t) -> p kt t", kt=4
                )
                for kin in range(4):
                    nc.sync.dma_start(out=t[:, kin, :], in_=xr[:, kin, :])
                ts_.append(t)
            return ts_

        x_next = load_x(0)
        for ic in range(N_HCH):
            x_sb = x_next

            # ---- A: Hx = H @ x per p-block ----
            hx = [None] * 16
            for kt in range(4):
                for p_ in range(4):
                    pa = ps.tile([P, HCHUNK], f32, tag="ta")
                    for kin in range(4):
                        nc.tensor.matmul(
                            pa, tf_sb[:, kin, kt, :], x_sb[p_][:, kin, :],
                            start=(kin == 0), stop=(kin == 3),
                        )
                    t_ = ap_.tile([P, HCHUNK], cdt, tag="act")
                    nc.scalar.copy(t_, pa)
                    hx[p_ * 4 + kt] = t_

            if ic + 1 < N_HCH:
                x_next = load_x(ic + 1)

            # ---- B: mix gate / up ----
            gH, uH = [], []
            for q in range(11):
                gq = ap_.tile([P, 4, HCHUNK], cdt, tag="act4", bufs=26)
                uq = ap_.tile([P, 4, HCHUNK], cdt, tag="act4", bufs=26)
                for kt in range(4):
                    o = q * 4 + kt
                    w_sb = mwp.tile([P, 2, 4, P], cdt, tag="mg")
                    mgu_o = mgu[o].rearrange("p (g a m) -> p g a m", g=2, a=4)
                    nc.sync.dma_start(out=w_sb[:, 0], in_=mgu_o[:, 0])
                    nc.sync.dma_start(out=w_sb[:, 1], in_=mgu_o[:, 1])
                    pb = ps.tile([P, HCHUNK], f32, tag="tb")
                    for p_ in range(4):
                        nc.tensor.matmul(
                            pb, w_sb[:, 0, p_, :], hx[p_ * 4 + kt],
                            start=(p_ == 0), stop=(p_ == 3),
                        )
                    nc.vector.tensor_copy(gq[:, kt, :], pb)

                    pb2 = ps.tile([P, HCHUNK], f32, tag="tb")
                    for p_ in range(4):
                        nc.tensor.matmul(
                            pb2, w_sb[:, 1, p_, :], hx[p_ * 4 + kt],
                            start=(p_ == 0), stop=(p_ == 3),
                        )
                    nc.vector.tensor_copy(uq[:, kt, :], pb2)
                gH.append(gq)
                uH.append(uq)

            # ---- C+D interleaved: real domain + SiLU*up, then Hh per q ----
            h = []
            hH = [None] * 11
            for q in range(11):
                h2q = ap_.tile([P, 4, HCHUNK], cdt, tag="act4", bufs=26)
                for mt in range(4):
                    pg = ps.tile([P, HCHUNK], f32, tag="tc")
                    for kt in range(4):
                        nc.tensor.matmul(
                            pg, ti_sb[:, kt, mt, :], gH[q][:, kt, :],
                            start=(kt == 0), stop=(kt == 3),
                        )
                    sg = sp.tile([P, HCHUNK], f32, tag="sg")
                    nc.scalar.activation(sg, pg, mybir.ActivationFunctionType.Silu)
                    pu = ps.tile([P, HCHUNK], f32, tag="tc")
                    for kt in range(4):
                        nc.tensor.matmul(
                            pu, ti_sb[:, kt, mt, :], uH[q][:, kt, :],
                            start=(kt == 0), stop=(kt == 3),
                        )
                    nc.vector.tensor_mul(h2q[:, mt, :], sg, pu)

                h.append(h2q)
                # Hh for this q (consumes h2q just produced)
                hq = ap_.tile([P, 4, HCHUNK], cdt, tag="act4", bufs=26)
                for kt in range(4):
                    pa = ps.tile([P, HCHUNK], f32, tag="ta")
                    for kin in range(4):
                        nc.tensor.matmul(
                            pa, tf_sb[:, kin, kt, :], h[q][:, kin, :],
                            start=(kin == 0), stop=(kin == 3),
                        )
                    if q % 2 == 0:
                        nc.scalar.copy(hq[:, kt, :], pa)
                    else:
                        nc.vector.tensor_copy(hq[:, kt, :], pa)
                hH[q] = hq


            # ---- E: mix down ----
            dH = []
            for p2 in range(4):
                dq = ap_.tile([P, 4, HCHUNK], cdt, tag="act4", bufs=26)
                for kt in range(4):
                    o = p2 * 4 + kt
                    wd_sb = mwp.tile([P, 11, P], cdt, tag="md")
                    md_o = md[o].rearrange("p (a m) -> p a m", a=11)
                    nc.sync.dma_start(out=wd_sb[:, :6], in_=md_o[:, :6])
                    nc.sync.dma_start(out=wd_sb[:, 6:], in_=md_o[:, 6:])
                    pb = ps.tile([P, HCHUNK], f32, tag="tb")
                    for q in range(11):
                        nc.tensor.matmul(
                            pb, wd_sb[:, q, :], hH[q][:, kt, :],
                            start=(q == 0), stop=(q == 10),
                        )
                    nc.vector.tensor_copy(dq[:, kt, :], pb)


                dH.append(dq)
                # ---- F: iH -> real output for this p2 ----
                for mt in range(4):
                    pf = ps.tile([P, HCHUNK], f32, tag="td")
                    for kt in range(4):
                        nc.tensor.matmul(
                            pf, ti_sb[:, kt, mt, :], dH[p2][:, kt, :],
                            start=(kt == 0), stop=(kt == 3),
                        )
                    o_sb = op.tile([P, HCHUNK], f32, tag="o")
                    nc.vector.tensor_copy(o_sb, pf)
                    nc.sync.dma_start(
                        out=out[p2 * 4 + mt][:, ic * HCHUNK:(ic + 1) * HCHUNK], in_=o_sb
                    )

    nc.finalize()
    _built["hart"] = nc
    return nc


QG = [(0, 1, 2, 3), (4, 5, 6, 7), (8, 9, 10)]  # q-groups for down contraction


# ---------------------------------------------------------------------------
# hart3: hart2 + quad unit-ordering + radix-4 two-stage C (iH) and D (H).
# The 512-pt Hartley transform factorizes as combine @ (I_4 (x) H_128) on
# time-decimated samples; the combine couples only the 4 frequencies sharing
# a sub-transform frequency (a "quad"), so with units ordered by quad it is
# 2x2-block-diagonal per 32-strip => col-tiled 4x-concurrent on the PE.
# C: 352 -> 440 MMs but 176 slots; D: 176 -> 352 MMs but ~90 slots.
# ---------------------------------------------------------------------------
def _quads():
    qs = [[0, 64, 128, 192]]
    for w in range(1, 64):
        qs.append([w, 128 - w, 128 + w, 256 - w])
    return qs


def _hart3_consts():
    quads = _quads()
    FR = np.zeros(512, np.int64)   # full row -> frequency
    for kt in range(4):
        for c in range(4):
            for t in range(16):
                u = quads[16 * c + t][kt]
                f0, f1 = (0, 256) if u == 0 else (u, 512 - u)
                FR[128 * kt + 32 * c + 2 * t + 0] = f0
                FR[128 * kt + 32 * c + 2 * t + 1] = f1
    RR = np.zeros(128, np.int64)   # sub row -> r
    for c in range(4):
        for t in range(16):
            w = 16 * c + t
            r0, r1 = (0, 64) if w == 0 else (w, 128 - w)
            RR[32 * c + 2 * t + 0] = r0
            RR[32 * c + 2 * t + 1] = r1

    def cas(n):
        f = np.arange(n)
        M = np.outer(f, f) * (2 * np.pi / n)
        return np.cos(M) + np.sin(M)

    H512, H128 = cas(512), cas(128)
    T_f = H512[FR].T.copy()              # [time, freq-row]
    T_i = (H512[FR] / 512.0).copy()      # [freq-row, time]
    SW_big = np.zeros((512, 512))
    for j in range(4):
        for sr in range(128):
            SW_big[4 * np.arange(128) + j, 128 * j + sr] = H128[RR[sr]]
    CW = np.linalg.solve(SW_big, T_f)          # [(j,subrow), freq-row]
    CW2 = np.linalg.solve(SW_big, T_i.T).T     # [freq-row, (j,subrow)]
    return quads, FR, RR, H128, T_f, T_i, CW, CW2


def _mix_blocks(w):
    """(q,p,512) circulant generators -> per-pair-unit 2x2 blocks [q,p,256,2,2]."""
    q, p, _ = w.shape
    W = np.fft.fft(w, axis=-1)
    Wh = (W.real - W.imag).astype(np.float64)
    fs = np.arange(256)
    gs = np.array([256] + [512 - u for u in range(1, 256)])
    Wp = (Wh[..., fs] + Wh[..., gs]) / 2
    Wm = (Wh[..., fs] - Wh[..., gs]) / 2
    blocks = np.zeros((q, p, 256, 2, 2))
    blocks[..., 0, 0] = Wp
    blocks[..., 0, 1] = -Wm
    blocks[..., 1, 0] = Wm
    blocks[..., 1, 1] = Wp
    blocks[..., 0, 0, 0] = Wh[..., 0]
    blocks[..., 0, 0, 1] = 0.0
    blocks[..., 0, 1, 0] = 0.0
    blocks[..., 0, 1, 1] = Wh[..., 256]
    return blocks


def _build_hart3():
    if "hart3" in _built:
        return _built["hart3"]
    cdt = mybir.dt.bfloat16
    f32 = mybir.dt.float32
    nc = bacc.Bacc("TRN2", debug=False, num_devices=N_CORES)

    xT = nc.dram_tensor("xT", [N_HCH, P, KT * HCHUNK], cdt, kind="ExternalInput").ap()
    tfa = nc.dram_tensor("tfa", [P, 2048], cdt, kind="ExternalInput").ap()
    tid = nc.dram_tensor("tid", [P, 2048], cdt, kind="ExternalInput").ap()
    wbd = nc.dram_tensor("wbd", [P, 11264], cdt, kind="ExternalInput").ap()
    wed = nc.dram_tensor("wed", [P, 6144], cdt, kind="ExternalInput").ap()
    icwd = nc.dram_tensor("icwd", [P, 512], cdt, kind="ExternalInput").ap()
    iswd = nc.dram_tensor("iswd", [P, 128], cdt, kind="ExternalInput").ap()
    swd = nc.dram_tensor("swd", [P, 128], cdt, kind="ExternalInput").ap()
    cwd = nc.dram_tensor("cwd", [P, 512], cdt, kind="ExternalInput").ap()
    out = nc.dram_tensor("outT", [KT, P, TOK_CORE], f32, kind="ExternalOutput").ap()

    H2 = 2 * HCHUNK

    with tile.TileContext(nc) as tc, ExitStack() as ctx:
        wpool = ctx.enter_context(tc.tile_pool(name="wpool", bufs=1))
        xp = ctx.enter_context(tc.tile_pool(name="xp", bufs=4))
        zxp = ctx.enter_context(tc.tile_pool(name="zxp", bufs=8))
        bop = ctx.enter_context(tc.tile_pool(name="bop", bufs=3))
        cmp_ = ctx.enter_context(tc.tile_pool(name="cmp", bufs=3))
        sgp = ctx.enter_context(tc.tile_pool(name="sgp", bufs=3))
        hp = ctx.enter_context(tc.tile_pool(name="hp", bufs=2))
        yhp = ctx.enter_context(tc.tile_pool(name="yhp", bufs=11))
        zhp = ctx.enter_context(tc.tile_pool(name="zhp", bufs=8))
        dp = ctx.enter_context(tc.tile_pool(name="dp", bufs=8))
        op = ctx.enter_context(tc.tile_pool(name="op", bufs=2))
        ps = ctx.enter_context(tc.tile_pool(name="ps", bufs=2, space="PSUM"))

        tfa_sb = wpool.tile([P, 4, 4, 4, 32], cdt, tag="tfa")
        ti_sb = wpool.tile([P, 4, 4, P], cdt, tag="ti")
        wb_sb = wpool.tile([P, 11, 4, 4, 2, 32], cdt, tag="wb")
        we_sb = wpool.tile([P, 4, 4, 4, 3, 32], cdt, tag="we")
        icw_sb = wpool.tile([P, 4, 4, 32], cdt, tag="icw")
        isw_sb = wpool.tile([P, P], cdt, tag="isw")
        sw_sb = wpool.tile([P, 4, 32], cdt, tag="sw")
        cw_sb = wpool.tile([P, 4, 4, 32], cdt, tag="cw")

        # warm the PE while the first DMAs are in flight
        wz = sgp.tile([P, HCHUNK], cdt, tag="wz", bufs=1)
        nc.vector.memset(wz, 0.0)
        wps = ps.tile([P, H2], f32, tag="tb")
        for wi in range(40):
            nc.tensor.matmul(wps[:, :HCHUNK], wz[:, :P], wz, start=(wi == 0), stop=(wi == 39))
        wdrain = sgp.tile([P, 4], f32, tag="wdrain", bufs=1)
        nc.vector.tensor_copy(wdrain, wps[:, :4])

        def load_x(ic):
            ts_ = []
            for p_ in range(4):
                t = xp.tile([P, 4, HCHUNK], cdt, tag="x", name=f"x{p_}")
                xr = xT[ic][:, p_ * 4 * HCHUNK:(p_ + 1) * 4 * HCHUNK].rearrange(
                    "p (kt t) -> p kt t", kt=4
                )
                for kin in range(4):
                    nc.sync.dma_start(out=t[:, kin, :], in_=xr[:, kin, :])
                ts_.append(t)
            return ts_

        x_next = load_x(0)
        nc.sync.dma_start(out=tfa_sb, in_=tfa.rearrange("p (a b c m) -> p a b c m", a=4, b=4, c=4))
        nc.sync.dma_start(out=ti_sb, in_=tid.rearrange("p (a b m) -> p a b m", a=4, b=4))
        nc.sync.dma_start(out=icw_sb, in_=icwd.rearrange("p (a b m) -> p a b m", a=4, b=4))
        nc.sync.dma_start(out=isw_sb, in_=iswd)
        nc.sync.dma_start(out=sw_sb, in_=swd.rearrange("p (a m) -> p a m", a=4))
        nc.sync.dma_start(out=cw_sb, in_=cwd.rearrange("p (a b m) -> p a b m", a=4, b=4))
        wb_r = wbd.rearrange("p (q a c g m) -> p q a c g m", q=11, a=4, c=4, g=2)
        for q2 in range(11):
            nc.sync.dma_start(out=wb_sb[:, q2], in_=wb_r[:, q2])
        nc.sync.dma_start(out=we_sb, in_=wed.rearrange("p (a b c d m) -> p a b c d m", a=4, b=4, c=4, d=3))

        def emit_A_tile(x_sb, kt, cp, zx):
            pa = ps.tile([P, H2], f32, tag="ta")
            for half in range(2):
                c = 2 * cp + half
                o0 = half * HCHUNK
                for kin in range(4):
                    for p_ in range(4):
                        nc.tensor.matmul(
                            pa[32 * p_:32 * p_ + 32, o0:o0 + HCHUNK],
                            tfa_sb[:, kin, kt, c, :], x_sb[p_][:, kin, :],
                            start=(kin == 0), stop=(kin == 3),
                            tile_position=(0, 32 * p_),
                        )
            t_ = zxp.tile([P, 2, HCHUNK], cdt, tag="zx")
            if (kt + cp) % 2 == 0:
                nc.scalar.copy(t_, pa)
            else:
                nc.vector.tensor_copy(t_, pa)
            zx[kt * 2 + cp] = t_

        # prologue: A for chunk 0 (not overlapped)
        zx_cur = [None] * 8
        for kt0 in range(4):
            for cp0 in range(2):
                emit_A_tile(x_next, kt0, cp0, zx_cur)

        for ic in range(N_HCH):
            if ic + 1 < N_HCH:
                x_next = load_x(ic + 1)
            zx = zx_cur

            def zx_at(kt, c):
                return zx[kt * 2 + c // 2][:, c % 2, :]

            # ---- stage emitters (software-pipelined q loop) ----
            def emit_B_gu(q, gu, dst):
                for cp0 in range(1):
                    for cp in range(2):
                        pb = ps.tile([P, H2], f32, tag="tb")
                        for half in range(2):
                            c = 2 * cp + half
                            o0 = half * HCHUNK
                            for kt in range(4):
                                nc.tensor.matmul(
                                    pb[32 * kt:32 * kt + 32, o0:o0 + HCHUNK],
                                    wb_sb[:, q, kt, c, gu, :], zx_at(kt, c),
                                    start=True, stop=True, tile_position=(0, 32 * kt),
                                )
                        if (q + cp + gu) % 2 == 0:
                            nc.vector.tensor_copy(dst[:, 2 * cp:2 * cp + 2, :], pb)
                        else:
                            nc.scalar.copy(dst[:, 2 * cp:2 * cp + 2, :], pb)

            def emit_Cc_gu(q, gu, src_, dst):
                for jp0 in range(1):
                    for jp in range(2):
                        pc = ps.tile([P, H2], f32, tag="tb")
                        for half in range(2):
                            j = 2 * jp + half
                            o0 = half * HCHUNK
                            for c in range(4):
                                nc.tensor.matmul(
                                    pc[32 * c:32 * c + 32, o0:o0 + HCHUNK],
                                    icw_sb[:, j, c, :], src_[:, c, :],
                                    start=True, stop=True, tile_position=(0, 32 * c),
                                )
                        if (q + jp + gu) % 2 == 0:
                            nc.scalar.copy(dst[:, 2 * jp:2 * jp + 2, :], pc)
                        else:
                            nc.vector.tensor_copy(dst[:, 2 * jp:2 * jp + 2, :], pc)

            def emit_C2_jp(q, cg, cu, hq, jp):
                for jp0 in range(1):
                    pg = ps.tile([P, H2], f32, tag="ta")
                    pu = ps.tile([P, H2], f32, tag="tb")
                    for half in range(2):
                        j = 2 * jp + half
                        o0 = half * HCHUNK
                        nc.tensor.matmul(pg[:, o0:o0 + HCHUNK], isw_sb, cg[:, j, :],
                                         start=True, stop=True)
                        nc.tensor.matmul(pu[:, o0:o0 + HCHUNK], isw_sb, cu[:, j, :],
                                         start=True, stop=True)
                    sg = sgp.tile([P, H2], cdt, tag="sg")
                    nc.scalar.activation(sg, pg, mybir.ActivationFunctionType.Silu)
                    nc.vector.tensor_mul(hq[:, 2 * jp:2 * jp + 2, :], sg, pu)

            def emit_D1_cp(q, hq, yh, cp):
                for cp0 in range(1):
                    pd = ps.tile([P, H2], f32, tag="ta")
                    for half in range(2):
                        c = 2 * cp + half
                        o0 = half * HCHUNK
                        for j in range(4):
                            nc.tensor.matmul(
                                pd[32 * j:32 * j + 32, o0:o0 + HCHUNK],
                                sw_sb[:, c, :], hq[:, j, :],
                                start=True, stop=True, tile_position=(0, 32 * j),
                            )
                    if cp == 0:
                        nc.scalar.copy(yh[:, 2 * cp:2 * cp + 2, :], pd)
                    else:
                        nc.vector.tensor_copy(yh[:, 2 * cp:2 * cp + 2, :], pd)

            # ---- pipelined B/Cc/C2/D1 over q ----
            bo_q = [None] * 11
            cm_q = [None] * 11
            h_q = [None] * 11
            yh_q = [None] * 11
            for qi in range(13):
                if qi < 11:
                    gq = bop.tile([P, 4, HCHUNK], cdt, tag="bo", name=f"gq{qi}")
                    uq = bop.tile([P, 4, HCHUNK], cdt, tag="bo", name=f"uq{qi}")
                    bo_q[qi] = (gq, uq)
                    emit_B_gu(qi, 0, gq)
                if 1 <= qi <= 11:
                    q = qi - 1
                    h_q[q] = hp.tile([P, 4, HCHUNK], cdt, tag="h", name=f"h{q}")
                    emit_C2_jp(q, cm_q[q][0], cm_q[q][1], h_q[q], 0)
                if qi < 11:
                    emit_B_gu(qi, 1, bo_q[qi][1])
                if 1 <= qi <= 11:
                    emit_C2_jp(qi - 1, cm_q[qi - 1][0], cm_q[qi - 1][1], h_q[qi - 1], 1)
                if qi < 11:
                    cg = cmp_.tile([P, 4, HCHUNK], cdt, tag="cm", name=f"cg{qi}")
                    cu = cmp_.tile([P, 4, HCHUNK], cdt, tag="cm", name=f"cu{qi}")
                    cm_q[qi] = (cg, cu)
                    emit_Cc_gu(qi, 0, bo_q[qi][0], cg)
                if 2 <= qi <= 12:
                    q = qi - 2
                    yh_q[q] = yhp.tile([P, 4, HCHUNK], cdt, tag="yh", name=f"yh{q}")
                    emit_D1_cp(q, h_q[q], yh_q[q], 0)
                if qi < 11:
                    emit_Cc_gu(qi, 1, bo_q[qi][1], cm_q[qi][1])
                if 2 <= qi <= 12:
                    q = qi - 2
                    emit_D1_cp(q, h_q[q], yh_q[q], 1)
                    h_q[q] = None
                    cm_q[q] = None
                    bo_q[q] = None

            # ---- D-combine + E interleaved per kt ----
            d_tiles = [None] * 8   # [p2pair*4 + kt] -> [P, 2, HCHUNK]

            def emit_Dc(kt, cs, zh):
                for c in cs:
                    zt = zhp.tile([P, 3, HCHUNK], cdt, tag="zh", name=f"zh{kt}{c}")
                    pz = ps.tile([P, H2], f32, tag="ta")
                    for half in range(2):
                        qgi = half
                        o0 = half * HCHUNK
                        for s, q in enumerate(QG[qgi]):
                            nc.tensor.matmul(
                                pz[32 * s:32 * s + 32, o0:o0 + HCHUNK],
                                cw_sb[:, kt, c, :], yh_q[q][:, c, :],
                                start=True, stop=True, tile_position=(0, 32 * s),
                            )
                    if c % 2 == 0:
                        nc.scalar.copy(zt[:, 0:2, :], pz)
                    else:
                        nc.vector.tensor_copy(zt[:, 0:2, :], pz)
                    pz2 = ps.tile([P, H2], f32, tag="ta")
                    for s, q in enumerate(QG[2]):
                        nc.tensor.matmul(
                            pz2[32 * s:32 * s + 32, 0:HCHUNK],
                            cw_sb[:, kt, c, :], yh_q[q][:, c, :],
                            start=True, stop=True, tile_position=(0, 32 * s),
                        )
                    if c % 2 == 0:
                        nc.vector.tensor_copy(zt[:, 2, :], pz2[:, 0:HCHUNK])
                    else:
                        nc.scalar.copy(zt[:, 2, :], pz2[:, 0:HCHUNK])
                    zh[c] = zt
                return zh

            def emit_E(kt, zh, pps):
                for pp in pps:
                    pe = ps.tile([P, H2], f32, tag="tb")
                    for half in range(2):
                        p2 = 2 * pp + half
                        o0 = half * HCHUNK
                        for qgi in range(3):
                            for c in range(4):
                                nc.tensor.matmul(
                                    pe[32 * c:32 * c + 32, o0:o0 + HCHUNK],
                                    we_sb[:, p2, kt, c, qgi, :], zh[c][:, qgi, :],
                                    start=(qgi == 0), stop=(qgi == 2),
                                    tile_position=(0, 32 * c),
                                )
                    t_ = dp.tile([P, 2, HCHUNK], cdt, tag="d")
                    if (kt + pp) % 2 == 0:
                        nc.vector.tensor_copy(t_, pe)
                    else:
                        nc.scalar.copy(t_, pe)
                    d_tiles[pp * 4 + kt] = t_

            zh_cur = [None] * 4
            emit_Dc(0, range(4), zh_cur)
            for kt in range(4):
                zh_nxt = [None] * 4
                if kt + 1 < 4:
                    emit_Dc(kt + 1, (0, 1), zh_nxt)
                emit_E(kt, zh_cur, (0,))
                if kt + 1 < 4:
                    emit_Dc(kt + 1, (2, 3), zh_nxt)
                emit_E(kt, zh_cur, (1,))
                zh_cur = zh_nxt

            # ---- F: dense iH -> output, interleaved with next chunk's A ----
            zx_nxt = [None] * 8
            fa_i = 0
            for p2 in range(4):
                for mp in range(2):
                    pf = ps.tile([P, H2], f32, tag="tb")
                    for half in range(2):
                        mt = 2 * mp + half
                        o0 = half * HCHUNK
                        for kt in range(4):
                            nc.tensor.matmul(
                                pf[:, o0:o0 + HCHUNK], ti_sb[:, kt, mt, :],
                                d_tiles[(p2 // 2) * 4 + kt][:, p2 % 2, :],
                                start=(kt == 0), stop=(kt == 3),
                            )
                    o_sb = op.tile([P, H2], f32, tag="o")
                    if (p2 + mp) % 2 == 0:
                        nc.scalar.copy(o_sb, pf)
                    else:
                        nc.vector.tensor_copy(o_sb, pf)
                    for half in range(2):
                        nc.sync.dma_start(
                            out=out[p2 * 4 + 2 * mp + half][:, ic * HCHUNK:(ic + 1) * HCHUNK],
                            in_=o_sb[:, half * HCHUNK:(half + 1) * HCHUNK],
                        )
                    if ic + 1 < N_HCH:
                        emit_A_tile(x_next, fa_i // 2, fa_i % 2, zx_nxt)
                    fa_i += 1
            zx_cur = zx_nxt

    nc.finalize()
    _built["hart3"] = nc
    return nc


def _hart3_in_maps(x, w_gate, w_up, w_down):
    quads, FR, RR, H128, T_f, T_i, CW, CW2 = _hart3_consts()
    bf = ml_dtypes.bfloat16
    U = np.zeros((4, 4, 16), np.int64)
    for kt in range(4):
        for c in range(4):
            for t in range(16):
                U[kt, c, t] = quads[16 * c + t][kt]

    tfa_pack = np.ascontiguousarray(
        T_f.reshape(4, P, 4, 4, 32).transpose(1, 0, 2, 3, 4)
    ).reshape(P, 2048).astype(bf)
    ti_pack = np.ascontiguousarray(
        T_i.reshape(4, P, 4, P).transpose(1, 0, 2, 3)
    ).reshape(P, 2048).astype(bf)

    # icw[j,c][32kt+i, a] = CW2[128kt+32c+i, 128j+32c+a]
    icw = np.zeros((4, 4, P, 32))
    for j in range(4):
        for c in range(4):
            for kt in range(4):
                rows = 128 * kt + 32 * c + np.arange(32)
                mids = 128 * j + 32 * c + np.arange(32)
                icw[j, c][32 * kt:32 * kt + 32] = CW2[np.ix_(rows, mids)]
    icw_pack = np.ascontiguousarray(icw.transpose(2, 0, 1, 3)).reshape(P, 512).astype(bf)
    isw_pack = H128[RR].astype(bf)

    sw = np.zeros((4, P, 32))
    for c in range(4):
        for idx in range(32):
            sw[c][:, idx] = H128[RR[32 * c + idx], :]
    sw_pack = np.ascontiguousarray(sw.transpose(1, 0, 2)).reshape(P, 128).astype(bf)

    cw = np.zeros((4, 4, P, 32))
    for kt in range(4):
        for c in range(4):
            rows_out = 128 * kt + 32 * c + np.arange(32)
            for j in range(4):
                mids = 128 * j + 32 * c + np.arange(32)
                cw[kt, c][32 * j:32 * j + 32] = CW[np.ix_(mids, rows_out)]
    cw_pack = np.ascontiguousarray(cw.transpose(2, 0, 1, 3)).reshape(P, 512).astype(bf)

    bg = _mix_blocks(np.asarray(w_gate, np.float64))
    bu = _mix_blocks(np.asarray(w_up, np.float64))
    bd = _mix_blocks(np.asarray(w_down, np.float64))

    # wb[32p+2t+b, q, kt, c, gu, 2t+a] = bl[q,p,U[kt,c,t],b,a]
    arr = np.stack([bg, bu], axis=0)           # [gu, q, p, unit, b, a]
    sel = arr[:, :, :, U]                      # [gu, q, p, kt, c, t, b, a]
    tmp = sel.transpose(2, 5, 6, 1, 3, 4, 0, 7)  # [p, t, b, q, kt, c, gu, a]
    wb = np.zeros((4, 16, 2, 11, 4, 4, 2, 16, 2))
    for t in range(16):
        wb[:, t, :, :, :, :, :, t, :] = tmp[:, t]
    wb_pack = wb.reshape(P, 11, 4, 4, 2, 32).reshape(P, 11264).astype(bf)

    # we[32s+2t+b, p2, kt, c, qgi, 2t+a] = bd[p2, QG[qgi][s], U[kt,c,t], b, a]
    seld = bd[:, :, U]                         # [p2, q, kt, c, t, b, a]
    we = np.zeros((4, 16, 2, 4, 4, 4, 3, 16, 2))
    for qgi, qs in enumerate(QG):
        for s, q in enumerate(qs):
            for t in range(16):
                # [p2, kt, c, b, a] -> [b, p2, kt, c, a]
                we[s, t, :, :, :, :, qgi, t, :] = seld[:, q, :, :, t].transpose(3, 0, 1, 2, 4)
    we_pack = we.reshape(P, 4, 4, 4, 3, 32).reshape(P, 6144).astype(bf)

    xf = np.asarray(x, np.float32).reshape(TOK_TOTAL, D_MODEL)
    in_maps = []
    for c0 in range(N_CORES):
        xc = xf[c0 * TOK_CORE:(c0 + 1) * TOK_CORE]
        xt = np.ascontiguousarray(
            xc.reshape(N_HCH, HCHUNK, KT, P).transpose(0, 3, 2, 1)
        ).reshape(N_HCH, P, KT * HCHUNK).astype(bf)
        in_maps.append({
            "xT": xt, "tfa": tfa_pack, "tid": ti_pack, "wbd": wb_pack,
            "wed": we_pack, "icwd": icw_pack, "iswd": isw_pack,
            "swd": sw_pack, "cwd": cw_pack,
        })
    return in_maps


def _build_hart2():
    """Col-tiled Hartley kernel: the 2x2-block-diagonal mix tiles are packed
    into 32-partition strips so 4 concurrent M=32 col-tiled matmuls replace 4
    serial 128x128 ones (mix stages run ~4x denser on the PE array).

    Per 512-token chunk (PE slots of ~216ns):
      A  x->packed hx      256 MMs (4x col-conc)  ~64 slots
      B  mix gate/up       352 MMs (4x col-conc)  ~88
      C  iH on gate/up     352 MMs dense           352
      D  h->packed Hh      704 MMs (4x col-conc) ~176
      E  mix down          192 MMs (4x col-conc)  ~48
      F  iH -> output       64 MMs dense            64
    total ~792 slots/chunk vs 1184 for mode "hart".
    """
    if "hart2" in _built:
        return _built["hart2"]
    cdt = mybir.dt.bfloat16
    f32 = mybir.dt.float32
    nc = bacc.Bacc("TRN2", debug=False, num_devices=N_CORES)

    xT = nc.dram_tensor("xT", [N_HCH, P, KT * HCHUNK], cdt, kind="ExternalInput").ap()
    tfa = nc.dram_tensor("tfa", [P, 4 * 4 * 4 * 32], cdt, kind="ExternalInput").ap()
    tid = nc.dram_tensor("tid", [P, 16 * P], cdt, kind="ExternalInput").ap()
    wb = nc.dram_tensor("wb", [P, 11 * 4 * 2 * 4 * 32], cdt, kind="ExternalInput").ap()
    we = nc.dram_tensor("we", [P, 4 * 4 * 4 * 3 * 32], cdt, kind="ExternalInput").ap()
    out = nc.dram_tensor("outT", [KT, P, TOK_CORE], f32, kind="ExternalOutput").ap()

    with tile.TileContext(nc) as tc, ExitStack() as ctx:
        wpool = ctx.enter_context(tc.tile_pool(name="wpool", bufs=1))
        xp = ctx.enter_context(tc.tile_pool(name="xp", bufs=8))
        zxp = ctx.enter_context(tc.tile_pool(name="zxp", bufs=8))
        gup = ctx.enter_context(tc.tile_pool(name="gup", bufs=4))
        sp = ctx.enter_context(tc.tile_pool(name="sp", bufs=2))
        hp = ctx.enter_context(tc.tile_pool(name="hp", bufs=11))
        zhp = ctx.enter_context(tc.tile_pool(name="zhp", bufs=24))
        dp = ctx.enter_context(tc.tile_pool(name="dp", bufs=17))
        op = ctx.enter_context(tc.tile_pool(name="op", bufs=2))
        ps = ctx.enter_context(tc.tile_pool(name="ps", bufs=2, space="PSUM"))

        tfa_sb = wpool.tile([P, 4, 4, 4, 32], cdt, tag="tfa")
        ti_sb = wpool.tile([P, 4, 4, P], cdt, tag="ti")
        wb_sb = wpool.tile([P, 11, 4, 2, 4, 32], cdt, tag="wb")
        we_sb = wpool.tile([P, 4, 4, 4, 3, 32], cdt, tag="we")

        # warm the PE while the first DMAs are in flight
        wz = sp.tile([P, HCHUNK], cdt, tag="wz", bufs=1)
        nc.vector.memset(wz, 0.0)
        wps = ps.tile([P, HCHUNK], f32, tag="td")
        for wi in range(40):
            nc.tensor.matmul(wps[:, :HCHUNK], wz[:, :P], wz, start=(wi == 0), stop=(wi == 39))
        wdrain = sp.tile([P, 4], f32, tag="wdrain", bufs=1)
        nc.vector.tensor_copy(wdrain, wps[:, :4])

        def load_x(ic):
            ts_ = []
            for p_ in range(4):
                t = xp.tile([P, 4, HCHUNK], cdt, tag="x")
                xr = xT[ic][:, p_ * 4 * HCHUNK:(p_ + 1) * 4 * HCHUNK].rearrange(
                    "p (kt t) -> p kt t", kt=4
                )
                for kin in range(4):
                    nc.sync.dma_start(out=t[:, kin, :], in_=xr[:, kin, :])
                ts_.append(t)
            return ts_

        x_next = load_x(0)
        nc.sync.dma_start(out=tfa_sb, in_=tfa.rearrange("p (a b c m) -> p a b c m", a=4, b=4, c=4))
        nc.sync.dma_start(out=ti_sb, in_=tid.rearrange("p (a b m) -> p a b m", a=4, b=4))
        wb_r = wb.rearrange("p (q a g c m) -> p q a g c m", q=11, a=4, g=2, c=4)
        for q2 in range(11):
            nc.sync.dma_start(out=wb_sb[:, q2], in_=wb_r[:, q2])
        nc.sync.dma_start(out=we_sb, in_=we.rearrange("p (a b c d m) -> p a b c d m", a=4, b=4, c=4, d=3))

        for ic in range(N_HCH):
            x_sb = x_next

            # ---- A: packed hx tiles Zx[kt,g] = [16 units(g) x 4 p-strips, T]
            zx = [None] * 16
            for kt in range(4):
                for g in range(4):
                    pa = ps.tile([P, HCHUNK], f32, tag="ta")
                    for kin in range(4):
                        for p_ in range(4):
                            nc.tensor.matmul(
                                pa[32 * p_:32 * p_ + 32, :],
                                tfa_sb[:, kin, kt, g, :], x_sb[p_][:, kin, :],
                                start=(kin == 0), stop=(kin == 3),
                                tile_position=(0, 32 * p_),
                            )
                    t_ = zxp.tile([P, HCHUNK], cdt, tag="zx")
                    nc.scalar.copy(t_, pa)
                    zx[kt * 4 + g] = t_

            if ic + 1 < N_HCH:
                x_next = load_x(ic + 1)

            # ---- B (col-tiled mix) + C (dense iH) interleaved per q ----
            def emit_B(q):
                gq = gup.tile([P, 4, HCHUNK], cdt, tag="gu")
                uq = gup.tile([P, 4, HCHUNK], cdt, tag="gu")
                for kt in range(4):
                    for gu, dst in ((0, gq), (1, uq)):
                        pb = ps.tile([P, HCHUNK], f32, tag="tb")
                        for g in range(4):
                            nc.tensor.matmul(
                                pb[32 * g:32 * g + 32, :],
                                wb_sb[:, q, kt, gu, g, :], zx[kt * 4 + g],
                                start=True, stop=True, tile_position=(0, 32 * g),
                            )
                        nc.vector.tensor_copy(dst[:, kt, :], pb)
                return gq, uq

            def emit_C(q, gq, uq, hq):
                for mt in range(4):
                    pg = ps.tile([P, HCHUNK], f32, tag="tc")
                    for kt in range(4):
                        nc.tensor.matmul(pg, ti_sb[:, kt, mt, :], gq[:, kt, :],
                                         start=(kt == 0), stop=(kt == 3))
                    sg = sp.tile([P, HCHUNK], f32, tag="sg")
                    nc.scalar.activation(sg, pg, mybir.ActivationFunctionType.Silu)
                    pu = ps.tile([P, HCHUNK], f32, tag="tc")
                    for kt in range(4):
                        nc.tensor.matmul(pu, ti_sb[:, kt, mt, :], uq[:, kt, :],
                                         start=(kt == 0), stop=(kt == 3))
                    nc.vector.tensor_mul(hq[:, mt, :], sg, pu)

            h = [hp.tile([P, 4, HCHUNK], cdt, tag="h", name=f"h{qi}") for qi in range(11)]
            cur = emit_B(0)
            for q in range(11):
                nxt = emit_B(q + 1) if q + 1 < 11 else None
                emit_C(q, cur[0], cur[1], h[q])
                cur = nxt

            # ---- D (h -> packed Hh) + E (mix down) interleaved per kt ----
            d_tiles = [None] * 16

            def emit_D(kt):
                zh = [None] * 12
                for g in range(4):
                    for qgi, qs in enumerate(QG):
                        pa = ps.tile([P, HCHUNK], f32, tag="ta")
                        for kin in range(4):
                            for s, q in enumerate(qs):
                                nc.tensor.matmul(
                                    pa[32 * s:32 * s + 32, :],
                                    tfa_sb[:, kin, kt, g, :], h[q][:, kin, :],
                                    start=(kin == 0), stop=(kin == 3),
                                    tile_position=(0, 32 * s),
                                )
                        t_ = zhp.tile([P, HCHUNK], cdt, tag="zh")
                        nc.scalar.copy(t_, pa)
                        zh[g * 3 + qgi] = t_
                return zh

            def emit_E(kt, zh):
                for p2 in range(4):
                    pe = ps.tile([P, HCHUNK], f32, tag="tb")
                    for qgi in range(3):
                        for g in range(4):
                            nc.tensor.matmul(
                                pe[32 * g:32 * g + 32, :],
                                we_sb[:, p2, kt, g, qgi, :], zh[g * 3 + qgi],
                                start=(qgi == 0), stop=(qgi == 2),
                                tile_position=(0, 32 * g),
                            )
                    t_ = dp.tile([P, HCHUNK], cdt, tag="d")
                    nc.vector.tensor_copy(t_, pe)
                    d_tiles[p2 * 4 + kt] = t_

            zh_cur = emit_D(0)
            for kt in range(4):
                zh_nxt = emit_D(kt + 1) if kt + 1 < 4 else None
                emit_E(kt, zh_cur)
                zh_cur = zh_nxt

            # ---- F: iH -> real output ----
            for p2 in range(4):
                for mt in range(4):
                    pf = ps.tile([P, HCHUNK], f32, tag="td")
                    for kt in range(4):
                        nc.tensor.matmul(pf, ti_sb[:, kt, mt, :], d_tiles[p2 * 4 + kt],
                                         start=(kt == 0), stop=(kt == 3))
                    o_sb = op.tile([P, HCHUNK], f32, tag="o")
                    nc.scalar.copy(o_sb, pf)
                    nc.sync.dma_start(
                        out=out[p2 * 4 + mt][:, ic * HCHUNK:(ic + 1) * HCHUNK], in_=o_sb
                    )

    nc.finalize()
    _built["hart2"] = nc
    return nc


def _hart2_in_maps(x, w_gate, w_up, w_down):
    T_f, T_i, R = _hartley_mats()
    bf = ml_dtypes.bfloat16

    tfa_pack = np.ascontiguousarray(
        T_f.reshape(4, P, 4, 4, 32).transpose(1, 0, 2, 3, 4)
    ).reshape(P, 4 * 4 * 4 * 32).astype(bf)
    ti_pack = np.ascontiguousarray(
        T_i.reshape(4, P, 4, P).transpose(1, 0, 2, 3)
    ).reshape(P, 16 * P).astype(bf)

    tg = _mix_tiles(np.asarray(w_gate, np.float32), R)   # [11,4,4,128,128]
    tu = _mix_tiles(np.asarray(w_up, np.float32), R)
    td = _mix_tiles(np.asarray(w_down, np.float32), R)   # [4,11,4,128,128]

    def diag_strips(t):  # [q,p,kt,128,128] -> [q,p,kt,g,32,32]
        return np.stack([t[..., 32 * g:32 * g + 32, 32 * g:32 * g + 32]
                         for g in range(4)], axis=3)

    gb = diag_strips(tg)  # [11,4(p),4(kt),4(g),32(r),32(c)]
    ub = diag_strips(tu)
    # wb[32p+r, q, kt, gu, g, c]
    wb_pack = np.stack([
        gb.transpose(1, 4, 0, 2, 3, 5).reshape(P, 11, 4, 4, 32),
        ub.transpose(1, 4, 0, 2, 3, 5).reshape(P, 11, 4, 4, 32),
    ], axis=3).reshape(P, 11 * 4 * 2 * 4 * 32).astype(bf)

    db = diag_strips(td)  # [4(p2),11(q),4(kt),4(g),32(r),32(c)]
    # we[32s+r, p2, kt, g, qgi, c]
    we_pack = np.zeros((4, 32, 4, 4, 4, 3, 32), np.float32)  # [s,r,p2,kt,g,qgi,c]
    for qgi, qs in enumerate(QG):
        for s, q in enumerate(qs):
            we_pack[s, :, :, :, :, qgi, :] = db[:, q].transpose(3, 0, 1, 2, 4)
    we_pack = we_pack.reshape(P, 4, 4, 4, 3, 32).reshape(P, -1).astype(bf)

    xf = np.asarray(x, np.float32).reshape(TOK_TOTAL, D_MODEL)
    in_maps = []
    for c in range(N_CORES):
        xc = xf[c * TOK_CORE:(c + 1) * TOK_CORE]
        xt = np.ascontiguousarray(
            xc.reshape(N_HCH, HCHUNK, KT, P).transpose(0, 3, 2, 1)
        ).reshape(N_HCH, P, KT * HCHUNK).astype(bf)
        in_maps.append({
            "xT": xt, "tfa": tfa_pack, "tid": ti_pack,
            "wb": wb_pack, "we": we_pack,
        })
    return in_maps


def _hart_in_maps(x, w_gate, w_up, w_down):
    T_f, T_i, R = _hartley_mats()
    bf = ml_dtypes.bfloat16

    tf_pack = np.ascontiguousarray(
        T_f.reshape(4, P, 4, P).transpose(1, 0, 2, 3)
    ).reshape(P, 16 * P).astype(bf)
    ti_pack = np.ascontiguousarray(
        T_i.reshape(4, P, 4, P).transpose(1, 0, 2, 3)
    ).reshape(P, 16 * P).astype(bf)

    tg = _mix_tiles(np.asarray(w_gate, np.float32), R)   # [11,4,4,128,128]
    tu = _mix_tiles(np.asarray(w_up, np.float32), R)
    td = _mix_tiles(np.asarray(w_down, np.float32), R)   # [4,11,4,128,128]
    mg_pack = np.ascontiguousarray(tg.transpose(0, 2, 3, 1, 4)).reshape(44, P, 4 * P)
    mu_pack = np.ascontiguousarray(tu.transpose(0, 2, 3, 1, 4)).reshape(44, P, 4 * P)
    mgu_pack = np.concatenate([mg_pack[:, :, None], mu_pack[:, :, None]], axis=2)
    mgu_pack = mgu_pack.reshape(44, P, 8 * P).astype(bf)
    md_pack = np.ascontiguousarray(td.transpose(0, 2, 3, 1, 4)).reshape(16, P, 11 * P).astype(bf)

    xf = np.asarray(x, np.float32).reshape(TOK_TOTAL, D_MODEL)
    in_maps = []
    for c in range(N_CORES):
        xc = xf[c * TOK_CORE:(c + 1) * TOK_CORE]
        xt = np.ascontiguousarray(
            xc.reshape(N_HCH, HCHUNK, KT, P).transpose(0, 3, 2, 1)
        ).reshape(N_HCH, P, KT * HCHUNK).astype(bf)
        in_maps.append({
            "xT": xt, "tf": tf_pack, "ti": ti_pack,
            "mgu": mgu_pack, "md": md_pack,
        })
    return in_maps


def _materialize(w):
    """(q, p, b) circulant generators -> dense [p*b, q*b] (in-dim, out-dim)."""
    q, p, b = w.shape
    i = np.arange(b)
    idx = (i[None, :] - i[:, None]) % b          # [j, i]
    return w[:, :, idx].transpose(1, 2, 0, 3).reshape(p * b, q * b)


def kernel(x, w_gate, w_up, w_down):
    mode = MODE
    if mode == "hart3":
        nc = _build_hart3()
        in_maps = _hart3_in_maps(x, w_gate, w_up, w_down)
        return _run(nc, in_maps)
    if mode == "hart2":
        nc = _build_hart2()
        in_maps = _hart2_in_maps(x, w_gate, w_up, w_down)
        return _run(nc, in_maps)
    if mode == "hart":
        nc = _build_hart()
        in_maps = _hart_in_maps(x, w_gate, w_up, w_down)
        return _run(nc, in_maps)
    cdt, npdt, pass_t, mm_n, _ = _MODE_CFG[mode]
    n_pass = TOK_CORE // pass_t

    nc = _build(mode)

    Wg = _materialize(np.asarray(w_gate, np.float32))   # [2048, 5632]
    Wu = _materialize(np.asarray(w_up, np.float32))     # [2048, 5632]
    Wd = _materialize(np.asarray(w_down, np.float32))   # [5632, 2048]

    # wgu packed: [MT, P, 2, KT, P]; per-partition rows contiguous
    wgu = np.empty((MT, P, 2, KT, P), np.float32)
    wg4 = Wg.reshape(KT, P, MT, P)   # [k, kp, m, mp]
    wu4 = Wu.reshape(KT, P, MT, P)
    wgu[:, :, 0] = wg4.transpose(2, 1, 0, 3)  # [m, kp, k, mp]
    wgu[:, :, 1] = wu4.transpose(2, 1, 0, 3)
    wgu = wgu.reshape(MT, P, 2 * KT * P).astype(npdt)

    wd4 = Wd.reshape(MT, P, KT, P)   # [k2, kp, m2, mp]
    wdp = np.ascontiguousarray(wd4.transpose(2, 1, 0, 3)).reshape(KT, P, MT * P).astype(npdt)

    xf = np.asarray(x, np.float32).reshape(TOK_TOTAL, D_MODEL)
    in_maps = []
    for c in range(N_CORES):
        xc = xf[c * TOK_CORE:(c + 1) * TOK_CORE]          # [2048 tok, 2048 d]
        # -> [n_pass, P, KT, pass_t]: xT[pass, kp, k, t] = xc[pass*pt+t, k*P+kp]
        xt = np.ascontiguousarray(
            xc.reshape(n_pass, pass_t, KT, P).transpose(0, 3, 2, 1)
        ).reshape(n_pass, P, KT * pass_t).astype(npdt)
        in_maps.append({"xT": xt, "wgu": wgu, "wd": wdp})

    return _run(nc, in_maps)


def _run(nc, in_maps):
    trace = bool(os.environ.get("BASS_PROFILE"))
    try:
        res = run_bass_kernel_spmd(nc, in_maps, core_ids=list(range(N_CORES)), trace=trace)
    except Exception:
        # transient device wedge (e.g. NRT_EXEC_UNIT_UNRECOVERABLE) - retry once
        import time as _t
        _t.sleep(5)
        res = run_bass_kernel_spmd(nc, in_maps, core_ids=list(range(N_CORES)), trace=trace)
    global last_results
    last_results = res

    out = np.empty((TOK_TOTAL, D_MODEL), np.float32)
    for c in range(N_CORES):
        o = res.results[c]["outT"]                         # [KT, P, TOK_CORE]
        out[c * TOK_CORE:(c + 1) * TOK_CORE] = o.reshape(D_MODEL, TOK_CORE).T
    return out.reshape(4, 4096, D_MODEL)



# revision 39
# speedup vs baseline: 1.2656x; 1.2656x over previous
"""Block-circulant SwiGLU feed-forward (CirculantFeedForward) for 8 trn2 cores.

Sharding: token-parallel across the 8 cores (16384 tokens -> 2048/core, no
collectives).  All weights are tiny circulant generators; host-side prep
turns them into matmul tiles packed in the exact SBUF layout.

Default mode "hart3" (~790 us HW, rel err 9.1e-3, vs 1074 us for the prior
"hart" baseline): block-circulant matmuls block-diagonalize in the real
Hartley basis as 2x2 pair blocks.  Two structural tricks on top of "hart":

1. Col-tiled mixes: frequency pair-units are packed so the 4-way block sums
   (over p for gate/up, over q-groups for down) live in the contract dim,
   and unit-groups map to 32-wide PE column strips.  Four concurrent M=32
   col-tiled matmuls (tile_position) replace four serial 128x128 ones, so
   each mix stage runs ~2.3-4x denser on the PE array.
2. Radix-4 two-stage transforms for the d_ff-sized directions: the 512-pt
   DHT factors as combine @ (I_4 (x) H_128) on time-decimated samples; with
   units ordered by "quad" (the 4 frequencies sharing a sub-transform
   frequency) the combine is itself a col-tiled sparse stage.  This halves
   the dense-transform matmul volume for the iH-after-mix (C) and
   H-before-downmix (D) stages.

Per 512-token chunk the PE slot count drops 1184 ("hart") -> ~610.  The
remaining gap to the slot model is LDWEIGHTS overhead on M=32 col-tiled
matmuls (~90ns/MM vs 54ns ideal) plus copy-latency coupling in the
B->Cc->C2 pipeline (PSUM evacuation on Scalar/Vector is the co-bottleneck).
Modes "hart2" (col-tiled mixes only), "hart", "bf16", "f32r", "f32" are
kept as fallbacks.
"""
import os
from contextlib import ExitStack

import numpy as np
import ml_dtypes

import concourse.bacc as bacc
import concourse.mybir as mybir
import concourse.tile as tile
from concourse.bass_utils import run_bass_kernel_spmd

N_CORES = 8
P = 128
B = 512
D_MODEL = 2048
D_FF = 5632
KT = D_MODEL // P    # 16 k-tiles (gate/up contraction; also down output tiles)
MT = D_FF // P       # 44 m-tiles (gate/up output; down contraction)
TOK_TOTAL = 16384
TOK_CORE = TOK_TOTAL // N_CORES  # 2048

MODE = os.environ.get("BASS_MODE", "hart3")  # hart3 | hart2 | hart | bf16 | f32 | f32r

_MODE_CFG = {
    # (mybir dtype, numpy dtype, tokens per pass, matmul N, wgu bufs)
    "bf16": (mybir.dt.bfloat16, ml_dtypes.bfloat16, 1024, 512, 3),
    "f32":  (mybir.dt.float32, np.float32, 512, 512, 2),
    "f32r": (mybir.dt.float32r, np.float32, 512, 512, 2),
}
KCH = 22  # down-weight chunk: MT=44 split into 2 chunks of 22 k-tiles

_built = {}
last_results = None

# ---------------------------------------------------------------------------
# Hartley (DHT) hybrid mode: block-circulant matmul block-diagonalizes in the
# real Hartley basis with (f, B-f) pairs interleaved.  Per 512-block:
#   y_q = iH @ sum_p Mix_qp @ (H @ x_p)
# where Mix_qp is 2x2-block-diagonal => its [128,128] tiles are diagonal.
# PE work per token drops from 2112 to 1184 matmul-tiles.
# ---------------------------------------------------------------------------
HCHUNK = 512   # tokens per chunk in hart mode
N_HCH = TOK_CORE // HCHUNK


def _hartley_mats():
    f = np.arange(B)
    M = np.outer(f, f) * (2 * np.pi / B)
    H = np.cos(M) + np.sin(M)
    order = [0, B // 2] + [v for u in range(1, B // 2) for v in (u, B - u)]
    R = np.array(order)
    T_f = (H[R].T).astype(np.float32)      # [in x, out hx]
    T_i = (H[R] / B).astype(np.float32)    # [in hx, out x]
    return T_f, T_i, R


def _mix_tiles(w, R):
    """w: (q, p, B) -> mix tiles [q, p, 4, 128, 128] ([in-row, out-col])."""
    q, p, _ = w.shape
    W = np.fft.fft(w, axis=-1)
    # DHT of w: W_h[f] = Re(W[f]) - Im(W[f])  (since cas = cos + sin)
    Wh = (W.real - W.imag).astype(np.float64)   # [q, p, B]
    fs = R[0::2]                                 # +side freq of each unit
    gs = R[1::2]
    Wp = (Wh[..., fs] + Wh[..., gs]) / 2         # [q, p, 256]
    Wm = (Wh[..., fs] - Wh[..., gs]) / 2
    # unit 0 is self-paired (f=0, g=B/2): block = diag(Wh[0], Wh[B/2])
    blocks = np.zeros((q, p, B // 2, 2, 2), np.float64)
    blocks[..., 0, 0] = Wp
    blocks[..., 0, 1] = -Wm
    blocks[..., 1, 0] = Wm
    blocks[..., 1, 1] = Wp
    blocks[..., 0, 0, 0] = Wh[..., 0]
    blocks[..., 0, 0, 1] = 0.0
    blocks[..., 0, 1, 0] = 0.0
    blocks[..., 0, 1, 1] = Wh[..., B // 2]
    # assemble [q, p, kt, 128, 128]: 64 units per kt-tile
    tiles = np.zeros((q, p, 4, P, P), np.float32)
    for u_lo in range(4):
        bb = blocks[:, :, u_lo * 64:(u_lo + 1) * 64]      # [q,p,64,2,2]
        t = np.zeros((q, p, 64, 2, 64, 2), np.float64)
        iu = np.arange(64)
        t[:, :, iu, :, iu, :] = bb.transpose(2, 0, 1, 3, 4)
        tiles[:, :, u_lo] = t.reshape(q, p, P, P)
    return tiles


def _build(mode):
    if mode in _built:
        return _built[mode]
    cdt, _, pass_t, mm_n, wgu_bufs = _MODE_CFG[mode]
    n_pass = TOK_CORE // pass_t
    n_nt = pass_t // mm_n  # matmul n-tiles per pass

    nc = bacc.Bacc("TRN2", debug=False, num_devices=N_CORES)
    f32 = mybir.dt.float32

    xT = nc.dram_tensor("xT", [n_pass, P, KT * pass_t], cdt, kind="ExternalInput").ap()
    wgu = nc.dram_tensor("wgu", [MT, P, 2 * KT * P], cdt, kind="ExternalInput").ap()
    wd = nc.dram_tensor("wd", [KT, P, MT * P], cdt, kind="ExternalInput").ap()
    out = nc.dram_tensor("outT", [KT, P, TOK_CORE], f32, kind="ExternalOutput").ap()

    with tile.TileContext(nc) as tc, ExitStack() as ctx:
        xp = ctx.enter_context(tc.tile_pool(name="xp", bufs=1))
        wp = ctx.enter_context(tc.tile_pool(name="wp", bufs=wgu_bufs))
        wdp_ = ctx.enter_context(tc.tile_pool(name="wdp", bufs=2))
        hp = ctx.enter_context(tc.tile_pool(name="hp", bufs=1))
        sp = ctx.enter_context(tc.tile_pool(name="sp", bufs=2))
        op = ctx.enter_context(tc.tile_pool(name="op", bufs=2))
        ps = ctx.enter_context(tc.tile_pool(name="ps", bufs=2, space="PSUM"))

        for ip in range(n_pass):
            x_sb = xp.tile([P, KT, pass_t], cdt, tag="x")
            nc.sync.dma_start(out=x_sb, in_=xT[ip].rearrange("p (kt t) -> p kt t", kt=KT))
            h_sb = hp.tile([P, MT, pass_t], cdt, tag="h")

            # ---- gate/up + SiLU*mul ----
            for m in range(MT):
                w_sb = wp.tile([P, 2, KT, P], cdt, tag="wgu")
                nc.sync.dma_start(
                    out=w_sb, in_=wgu[m].rearrange("p (g kt q) -> p g kt q", g=2, kt=KT)
                )
                pg = ps.tile([P, pass_t], f32, tag="a")
                pu = ps.tile([P, pass_t], f32, tag="b")
                for k in range(KT):
                    for j in range(n_nt):
                        nc.tensor.matmul(
                            pg[:, j * mm_n:(j + 1) * mm_n], w_sb[:, 0, k, :],
                            x_sb[:, k, j * mm_n:(j + 1) * mm_n],
                            start=(k == 0), stop=(k == KT - 1),
                        )
                    for j in range(n_nt):
                        nc.tensor.matmul(
                            pu[:, j * mm_n:(j + 1) * mm_n], w_sb[:, 1, k, :],
                            x_sb[:, k, j * mm_n:(j + 1) * mm_n],
                            start=(k == 0), stop=(k == KT - 1),
                        )
                sg = sp.tile([P, pass_t], f32, tag="sg")
                nc.scalar.activation(sg, pg, mybir.ActivationFunctionType.Silu)
                nc.vector.tensor_mul(h_sb[:, m, :], sg, pu)

            # ---- down ----
            for m2 in range(KT):
                pd = ps.tile([P, pass_t], f32, tag="a")
                for ch in range(MT // KCH):
                    wd_sb = wdp_.tile([P, KCH, P], cdt, tag="wd")
                    nc.sync.dma_start(
                        out=wd_sb,
                        in_=wd[m2][:, ch * KCH * P:(ch + 1) * KCH * P].rearrange(
                            "p (kt q) -> p kt q", kt=KCH
                        ),
                    )
                    for kc in range(KCH):
                        k2 = ch * KCH + kc
                        for j in range(n_nt):
                            nc.tensor.matmul(
                                pd[:, j * mm_n:(j + 1) * mm_n], wd_sb[:, kc, :],
                                h_sb[:, k2, j * mm_n:(j + 1) * mm_n],
                                start=(k2 == 0), stop=(k2 == MT - 1),
                            )
                o_sb = op.tile([P, pass_t], f32, tag="o")
                nc.vector.tensor_copy(o_sb, pd)
                nc.sync.dma_start(
                    out=out[m2][:, ip * pass_t:(ip + 1) * pass_t], in_=o_sb
                )

    nc.finalize()
    _built[mode] = nc
    return nc


def _build_hart():
    if "hart" in _built:
        return _built["hart"]
    cdt = mybir.dt.bfloat16
    f32 = mybir.dt.float32
    nc = bacc.Bacc("TRN2", debug=False, num_devices=N_CORES)

    xT = nc.dram_tensor("xT", [N_HCH, P, KT * HCHUNK], cdt, kind="ExternalInput").ap()
    tf = nc.dram_tensor("tf", [P, 16 * P], cdt, kind="ExternalInput").ap()
    ti = nc.dram_tensor("ti", [P, 16 * P], cdt, kind="ExternalInput").ap()
    mgu = nc.dram_tensor("mgu", [44, P, 8 * P], cdt, kind="ExternalInput").ap()
    md = nc.dram_tensor("md", [16, P, 11 * P], cdt, kind="ExternalInput").ap()
    out = nc.dram_tensor("outT", [KT, P, TOK_CORE], f32, kind="ExternalOutput").ap()

    with tile.TileContext(nc) as tc, ExitStack() as ctx:
        xp = ctx.enter_context(tc.tile_pool(name="xp", bufs=8))
        tp = ctx.enter_context(tc.tile_pool(name="tp", bufs=1))
        mwp = ctx.enter_context(tc.tile_pool(name="mwp", bufs=4))
        ap_ = ctx.enter_context(tc.tile_pool(name="ap", bufs=24))
        sp = ctx.enter_context(tc.tile_pool(name="sp", bufs=2))
        op = ctx.enter_context(tc.tile_pool(name="op", bufs=2))
        ps = ctx.enter_context(tc.tile_pool(name="ps", bufs=2, space="PSUM"))
        ps3 = ctx.enter_context(tc.tile_pool(name="ps3", bufs=3, space="PSUM"))
        ps1 = ctx.enter_context(tc.tile_pool(name="ps1", bufs=1, space="PSUM"))

        tf_sb = tp.tile([P, 4, 4, P], cdt, tag="tf")
        ti_sb = tp.tile([P, 4, 4, P], cdt, tag="ti")
        # warm the PE (HAM un-throttle) while the first DMAs are in flight
        wz = ap_.tile([P, HCHUNK], cdt, tag="act")
        nc.vector.memset(wz, 0.0)
        wps = ps.tile([P, H2], f32, tag="tb")
        for wi in range(20):
            nc.tensor.matmul(wz_ := wps, wz[:, :P], wz, start=(wi == 0), stop=(wi == 19))
        wdrain = ap_.tile([P, 4], f32, tag="wdrain")
        nc.vector.tensor_copy(wdrain, wps[:, :4])
        tf_r = tf.rearrange("p (a b m) -> p a b m", a=4, b=4)
        for b_ in range(4):
            nc.sync.dma_start(out=tf_sb[:, :, b_], in_=tf_r[:, :, b_])
        nc.sync.dma_start(out=ti_sb, in_=ti.rearrange("p (a b m) -> p a b m", a=4, b=4))

        def load_x(ic):
            ts_ = []
            for p_ in range(4):
                t = xp.tile([P, 4, HCHUNK], cdt, tag="x")
                xr = xT[ic][:, p_ * 4 * HCHUNK:(p_ + 1) * 4 * HCHUNK].rearrange(
                    "p (kt t) -> p kt t", kt=4
                )
                for kin in range(4):
                    nc.sync.dma_start(out=t[:, kin, :], in_=xr[:, kin, :])
                ts_.append(t)
            return ts_

        x_next = load_x(0)
        for ic in range(N_HCH):
            x_sb = x_next

            # ---- A: Hx = H @ x per p-block ----
            hx = [None] * 16
            for kt in range(4):
                for p_ in range(4):
                    pa = ps.tile([P, HCHUNK], f32, tag="ta")
                    for kin in range(4):
                        nc.tensor.matmul(
                            pa, tf_sb[:, kin, kt, :], x_sb[p_][:, kin, :],
                            start=(kin == 0), stop=(kin == 3),
                        )
                    t_ = ap_.tile([P, HCHUNK], cdt, tag="act")
                    nc.scalar.copy(t_, pa)
                    hx[p_ * 4 + kt] = t_

            if ic + 1 < N_HCH:
                x_next = load_x(ic + 1)

            # ---- B: mix gate / up ----
            gH, uH = [], []
            for q in range(11):
                gq = ap_.tile([P, 4, HCHUNK], cdt, tag="act4", bufs=26)
                uq = ap_.tile([P, 4, HCHUNK], cdt, tag="act4", bufs=26)
                for kt in range(4):
                    o = q * 4 + kt
                    w_sb = mwp.tile([P, 2, 4, P], cdt, tag="mg")
                    mgu_o = mgu[o].rearrange("p (g a m) -> p g a m", g=2, a=4)
                    nc.sync.dma_start(out=w_sb[:, 0], in_=mgu_o[:, 0])
                    nc.sync.dma_start(out=w_sb[:, 1], in_=mgu_o[:, 1])
                    pb = ps.tile([P, HCHUNK], f32, tag="tb")
                    for p_ in range(4):
                        nc.tensor.matmul(
                            pb, w_sb[:, 0, p_, :], hx[p_ * 4 + kt],
                            start=(p_ == 0), stop=(p_ == 3),
                        )
                    nc.vector.tensor_copy(gq[:, kt, :], pb)

                    pb2 = ps.tile([P, HCHUNK], f32, tag="tb")
                    for p_ in range(4):
                        nc.tensor.matmul(
                            pb2, w_sb[:, 1, p_, :], hx[p_ * 4 + kt],
                            start=(p_ == 0), stop=(p_ == 3),
                        )
                    nc.vector.tensor_copy(uq[:, kt, :], pb2)
                gH.append(gq)
                uH.append(uq)

            # ---- C+D interleaved: real domain + SiLU*up, then Hh per q ----
            h = []
            hH = [None] * 11
            for q in range(11):
                h2q = ap_.tile([P, 4, HCHUNK], cdt, tag="act4", bufs=26)
                for mt in range(4):
                    pg = ps.tile([P, HCHUNK], f32, tag="tc")
                    for kt in range(4):
                        nc.tensor.matmul(
                            pg, ti_sb[:, kt, mt, :], gH[q][:, kt, :],
                            start=(kt == 0), stop=(kt == 3),
                        )
                    sg = sp.tile([P, HCHUNK], f32, tag="sg")
                    nc.scalar.activation(sg, pg, mybir.ActivationFunctionType.Silu)
                    pu = ps.tile([P, HCHUNK], f32, tag="tc")
                    for kt in range(4):
                        nc.tensor.matmul(
                            pu, ti_sb[:, kt, mt, :], uH[q][:, kt, :],
                            start=(kt == 0), stop=(kt == 3),
                        )
                    nc.vector.tensor_mul(h2q[:, mt, :], sg, pu)

                h.append(h2q)
                # Hh for this q (consumes h2q just produced)
                hq = ap_.tile([P, 4, HCHUNK], cdt, tag="act4", bufs=26)
                for kt in range(4):
                    pa = ps.tile([P, HCHUNK], f32, tag="ta")
                    for kin in range(4):
                        nc.tensor.matmul(
                            pa, tf_sb[:, kin, kt, :], h[q][:, kin, :],
                            start=(kin == 0), stop=(kin == 3),
                        )
                    if q % 2 == 0:
                        nc.scalar.copy(hq[:, kt, :], pa)
                    else:
                        nc.vector.tensor_copy(hq[:, kt, :], pa)
                hH[q] = hq


            # ---- E: mix down ----
            dH = []
            for p2 in range(4):
                dq = ap_.tile([P, 4, HCHUNK], cdt, tag="act4", bufs=26)
                for kt in range(4):
                    o = p2 * 4 + kt
                    wd_sb = mwp.tile([P, 11, P], cdt, tag="md")
                    md_o = md[o].rearrange("p (a m) -> p a m", a=11)
                    nc.sync.dma_start(out=wd_sb[:, :6], in_=md_o[:, :6])
                    nc.sync.dma_start(out=wd_sb[:, 6:], in_=md_o[:, 6:])
                    pb = ps.tile([P, HCHUNK], f32, tag="tb")
                    for q in range(11):
                        nc.tensor.matmul(
                            pb, wd_sb[:, q, :], hH[q][:, kt, :],
                            start=(q == 0), stop=(q == 10),
                        )
                    nc.vector.tensor_copy(dq[:, kt, :], pb)


                dH.append(dq)
                # ---- F: iH -> real output for this p2 ----
                for mt in range(4):
                    pf = ps.tile([P, HCHUNK], f32, tag="td")
                    for kt in range(4):
                        nc.tensor.matmul(
                            pf, ti_sb[:, kt, mt, :], dH[p2][:, kt, :],
                            start=(kt == 0), stop=(kt == 3),
                        )
                    o_sb = op.tile([P, HCHUNK], f32, tag="o")
                    nc.vector.tensor_copy(o_sb, pf)
                    nc.sync.dma_start(
                        out=out[p2 * 4 + mt][:, ic * HCHUNK:(ic + 1) * HCHUNK], in_=o_sb
                    )

    nc.finalize()
    _built["hart"] = nc
    return nc


QG = [(0, 1, 2, 3), (4, 5, 6, 7), (8, 9, 10)]  # q-groups for down contraction


# ---------------------------------------------------------------------------
# hart3: hart2 + quad unit-ordering + radix-4 two-stage C (iH) and D (H).
# The 512-pt Hartley transform factorizes as combine @ (I_4 (x) H_128) on
# time-decimated samples; the combine couples only the 4 frequencies sharing
# a sub-transform frequency (a "quad"), so with units ordered by quad it is
# 2x2-block-diagonal per 32-strip => col-tiled 4x-concurrent on the PE.
# C: 352 -> 440 MMs but 176 slots; D: 176 -> 352 MMs but ~90 slots.
# ---------------------------------------------------------------------------
def _quads():
    qs = [[0, 64, 128, 192]]
    for w in range(1, 64):
        qs.append([w, 128 - w, 128 + w, 256 - w])
    return qs


def _hart3_consts():
    quads = _quads()
    FR = np.zeros(512, np.int64)   # full row -> frequency
    for kt in range(4):
        for c in range(4):
            for t in range(16):
                u = quads[16 * c + t][kt]
                f0, f1 = (0, 256) if u == 0 else (u, 512 - u)
                FR[128 * kt + 32 * c + 2 * t + 0] = f0
                FR[128 * kt + 32 * c + 2 * t + 1] = f1
    RR = np.zeros(128, np.int64)   # sub row -> r
    for c in range(4):
        for t in range(16):
            w = 16 * c + t
            r0, r1 = (0, 64) if w == 0 else (w, 128 - w)
            RR[32 * c + 2 * t + 0] = r0
            RR[32 * c + 2 * t + 1] = r1

    def cas(n):
        f = np.arange(n)
        M = np.outer(f, f) * (2 * np.pi / n)
        return np.cos(M) + np.sin(M)

    H512, H128 = cas(512), cas(128)
    T_f = H512[FR].T.copy()              # [time, freq-row]
    T_i = (H512[FR] / 512.0).copy()      # [freq-row, time]
    SW_big = np.zeros((512, 512))
    for j in range(4):
        for sr in range(128):
            SW_big[4 * np.arange(128) + j, 128 * j + sr] = H128[RR[sr]]
    CW = np.linalg.solve(SW_big, T_f)          # [(j,subrow), freq-row]
    CW2 = np.linalg.solve(SW_big, T_i.T).T     # [freq-row, (j,subrow)]
    return quads, FR, RR, H128, T_f, T_i, CW, CW2


def _mix_blocks(w):
    """(q,p,512) circulant generators -> per-pair-unit 2x2 blocks [q,p,256,2,2]."""
    q, p, _ = w.shape
    W = np.fft.fft(w, axis=-1)
    Wh = (W.real - W.imag).astype(np.float64)
    fs = np.arange(256)
    gs = np.array([256] + [512 - u for u in range(1, 256)])
    Wp = (Wh[..., fs] + Wh[..., gs]) / 2
    Wm = (Wh[..., fs] - Wh[..., gs]) / 2
    blocks = np.zeros((q, p, 256, 2, 2))
    blocks[..., 0, 0] = Wp
    blocks[..., 0, 1] = -Wm
    blocks[..., 1, 0] = Wm
    blocks[..., 1, 1] = Wp
    blocks[..., 0, 0, 0] = Wh[..., 0]
    blocks[..., 0, 0, 1] = 0.0
    blocks[..., 0, 1, 0] = 0.0
    blocks[..., 0, 1, 1] = Wh[..., 256]
    return blocks


def _build_hart3():
    if "hart3" in _built:
        return _built["hart3"]
    cdt = mybir.dt.bfloat16
    f32 = mybir.dt.float32
    nc = bacc.Bacc("TRN2", debug=False, num_devices=N_CORES)

    xT = nc.dram_tensor("xT", [N_HCH, P, KT * HCHUNK], cdt, kind="ExternalInput").ap()
    tfa = nc.dram_tensor("tfa", [P, 2048], cdt, kind="ExternalInput").ap()
    tid = nc.dram_tensor("tid", [P, 2048], cdt, kind="ExternalInput").ap()
    wbd = nc.dram_tensor("wbd", [P, 11264], cdt, kind="ExternalInput").ap()
    wed = nc.dram_tensor("wed", [P, 6144], cdt, kind="ExternalInput").ap()
    icwd = nc.dram_tensor("icwd", [P, 512], cdt, kind="ExternalInput").ap()
    iswd = nc.dram_tensor("iswd", [P, 128], cdt, kind="ExternalInput").ap()
    swd = nc.dram_tensor("swd", [P, 128], cdt, kind="ExternalInput").ap()
    cwd = nc.dram_tensor("cwd", [P, 512], cdt, kind="ExternalInput").ap()
    out = nc.dram_tensor("outT", [KT, P, TOK_CORE], f32, kind="ExternalOutput").ap()

    H2 = 2 * HCHUNK

    with tile.TileContext(nc) as tc, ExitStack() as ctx:
        wpool = ctx.enter_context(tc.tile_pool(name="wpool", bufs=1))
        xp = ctx.enter_context(tc.tile_pool(name="xp", bufs=4))
        zxp = ctx.enter_context(tc.tile_pool(name="zxp", bufs=8))
        bop = ctx.enter_context(tc.tile_pool(name="bop", bufs=3))
        cmp_ = ctx.enter_context(tc.tile_pool(name="cmp", bufs=3))
        sgp = ctx.enter_context(tc.tile_pool(name="sgp", bufs=3))
        hp = ctx.enter_context(tc.tile_pool(name="hp", bufs=2))
        yhp = ctx.enter_context(tc.tile_pool(name="yhp", bufs=11))
        zhp = ctx.enter_context(tc.tile_pool(name="zhp", bufs=8))
        dp = ctx.enter_context(tc.tile_pool(name="dp", bufs=8))
        op = ctx.enter_context(tc.tile_pool(name="op", bufs=2))
        ps = ctx.enter_context(tc.tile_pool(name="ps", bufs=2, space="PSUM"))

        tfa_sb = wpool.tile([P, 4, 4, 4, 32], cdt, tag="tfa")
        ti_sb = wpool.tile([P, 4, 4, P], cdt, tag="ti")
        wb_sb = wpool.tile([P, 11, 4, 4, 2, 32], cdt, tag="wb")
        we_sb = wpool.tile([P, 4, 4, 4, 3, 32], cdt, tag="we")
        icw_sb = wpool.tile([P, 4, 4, 32], cdt, tag="icw")
        isw_sb = wpool.tile([P, P], cdt, tag="isw")
        sw_sb = wpool.tile([P, 4, 32], cdt, tag="sw")
        cw_sb = wpool.tile([P, 4, 4, 32], cdt, tag="cw")

        # warm the PE while the first DMAs are in flight
        wz = sgp.tile([P, HCHUNK], cdt, tag="wz", bufs=1)
        nc.vector.memset(wz, 0.0)
        wps = ps.tile([P, H2], f32, tag="tb")
        for wi in range(40):
            nc.tensor.matmul(wps[:, :HCHUNK], wz[:, :P], wz, start=(wi == 0), stop=(wi == 39))
        wdrain = sgp.tile([P, 4], f32, tag="wdrain", bufs=1)
        nc.vector.tensor_copy(wdrain, wps[:, :4])

        def load_x(ic):
            ts_ = []
            for p_ in range(4):
                t = xp.tile([P, 4, HCHUNK], cdt, tag="x", name=f"x{p_}")
                xr = xT[ic][:, p_ * 4 * HCHUNK:(p_ + 1) * 4 * HCHUNK].rearrange(
                    "p (kt t) -> p kt t", kt=4
                )
                for kin in range(4):
                    nc.sync.dma_start(out=t[:, kin, :], in_=xr[:, kin, :])
                ts_.append(t)
            return ts_

        x_next = load_x(0)
        nc.sync.dma_start(out=tfa_sb, in_=tfa.rearrange("p (a b c m) -> p a b c m", a=4, b=4, c=4))
        nc.sync.dma_start(out=ti_sb, in_=tid.rearrange("p (a b m) -> p a b m", a=4, b=4))
        nc.sync.dma_start(out=icw_sb, in_=icwd.rearrange("p (a b m) -> p a b m", a=4, b=4))
        nc.sync.dma_start(out=isw_sb, in_=iswd)
        nc.sync.dma_start(out=sw_sb, in_=swd.rearrange("p (a m) -> p a m", a=4))
        nc.sync.dma_start(out=cw_sb, in_=cwd.rearrange("p (a b m) -> p a b m", a=4, b=4))
        wb_r = wbd.rearrange("p (q a c g m) -> p q a c g m", q=11, a=4, c=4, g=2)
        for q2 in range(11):
            nc.sync.dma_start(out=wb_sb[:, q2], in_=wb_r[:, q2])
        nc.sync.dma_start(out=we_sb, in_=wed.rearrange("p (a b c d m) -> p a b c d m", a=4, b=4, c=4, d=3))

        def emit_A_tile(x_sb, kt, cp, zx):
            pa = ps.tile([P, H2], f32, tag="ta")
            for half in range(2):
                c = 2 * cp + half
                o0 = half * HCHUNK
                for kin in range(4):
                    for p_ in range(4):
                        nc.tensor.matmul(
                            pa[32 * p_:32 * p_ + 32, o0:o0 + HCHUNK],
                            tfa_sb[:, kin, kt, c, :], x_sb[p_][:, kin, :],
                            start=(kin == 0), stop=(kin == 3),
                            tile_position=(0, 32 * p_),
                        )
            t_ = zxp.tile([P, 2, HCHUNK], cdt, tag="zx")
            if (kt + cp) % 2 == 0:
                nc.scalar.copy(t_, pa)
            else:
                nc.vector.tensor_copy(t_, pa)
            zx[kt * 2 + cp] = t_

        # prologue: A for chunk 0 (not overlapped)
        zx_cur = [None] * 8
        for kt0 in range(4):
            for cp0 in range(2):
                emit_A_tile(x_next, kt0, cp0, zx_cur)

        for ic in range(N_HCH):
            if ic + 1 < N_HCH:
                x_next = load_x(ic + 1)
            zx = zx_cur

            def zx_at(kt, c):
                return zx[kt * 2 + c // 2][:, c % 2, :]

            # ---- stage emitters (software-pipelined q loop) ----
            def emit_B_gu(q, gu, dst):
                for cp0 in range(1):
                    for cp in range(2):
                        pb = ps.tile([P, H2], f32, tag="tb")
                        for half in range(2):
                            c = 2 * cp + half
                            o0 = half * HCHUNK
                            for kt in range(4):
                                nc.tensor.matmul(
                                    pb[32 * kt:32 * kt + 32, o0:o0 + HCHUNK],
                                    wb_sb[:, q, kt, c, gu, :], zx_at(kt, c),
                                    start=True, stop=True, tile_position=(0, 32 * kt),
                                )
                        if (q + cp + gu) % 2 == 0:
                            nc.vector.tensor_copy(dst[:, 2 * cp:2 * cp + 2, :], pb)
                        else:
                            nc.scalar.copy(dst[:, 2 * cp:2 * cp + 2, :], pb)

            def emit_Cc_gu(q, gu, src_, dst):
                for jp0 in range(1):
                    for jp in range(2):
                        pc = ps.tile([P, H2], f32, tag="tb")
                        for half in range(2):
                            j = 2 * jp + half
                            o0 = half * HCHUNK
                            for c in range(4):
                                nc.tensor.matmul(
                                    pc[32 * c:32 * c + 32, o0:o0 + HCHUNK],
                                    icw_sb[:, j, c, :], src_[:, c, :],
                                    start=True, stop=True, tile_position=(0, 32 * c),
                                )
                        if (q + jp + gu) % 2 == 0:
                            nc.scalar.copy(dst[:, 2 * jp:2 * jp + 2, :], pc)
                        else:
                            nc.vector.tensor_copy(dst[:, 2 * jp:2 * jp + 2, :], pc)

            def emit_C2_jp(q, cg, cu, hq, jp):
                for jp0 in range(1):
                    pg = ps.tile([P, H2], f32, tag="ta")
                    pu = ps.tile([P, H2], f32, tag="ta")
                    for half in range(2):
                        j = 2 * jp + half
                        o0 = half * HCHUNK
                        nc.tensor.matmul(pg[:, o0:o0 + HCHUNK], isw_sb, cg[:, j, :],
                                         start=True, stop=True)
                        nc.tensor.matmul(pu[:, o0:o0 + HCHUNK], isw_sb, cu[:, j, :],
                                         start=True, stop=True)
                    sg = sgp.tile([P, H2], cdt, tag="sg")
                    nc.scalar.activation(sg, pg, mybir.ActivationFunctionType.Silu)
                    nc.vector.tensor_mul(hq[:, 2 * jp:2 * jp + 2, :], sg, pu)

            def emit_D1_cp(q, hq, yh, cp):
                for cp0 in range(1):
                    pd = ps.tile([P, H2], f32, tag="ta")
                    for half in range(2):
                        c = 2 * cp + half
                        o0 = half * HCHUNK
                        for j in range(4):
                            nc.tensor.matmul(
                                pd[32 * j:32 * j + 32, o0:o0 + HCHUNK],
                                sw_sb[:, c, :], hq[:, j, :],
                                start=True, stop=True, tile_position=(0, 32 * j),
                            )
                    if cp == 0:
                        nc.scalar.copy(yh[:, 2 * cp:2 * cp + 2, :], pd)
                    else:
                        nc.vector.tensor_copy(yh[:, 2 * cp:2 * cp + 2, :], pd)

            # ---- pipelined B/Cc/C2/D1 over q ----
            bo_q = [None] * 11
            cm_q = [None] * 11
            h_q = [None] * 11
            yh_q = [None] * 11
            for qi in range(13):
                if qi < 11:
                    gq = bop.tile([P, 4, HCHUNK], cdt, tag="bo", name=f"gq{qi}")
                    uq = bop.tile([P, 4, HCHUNK], cdt, tag="bo", name=f"uq{qi}")
                    bo_q[qi] = (gq, uq)
                    emit_B_gu(qi, 0, gq)
                if 1 <= qi <= 11:
                    q = qi - 1
                    h_q[q] = hp.tile([P, 4, HCHUNK], cdt, tag="h", name=f"h{q}")
                    emit_C2_jp(q, cm_q[q][0], cm_q[q][1], h_q[q], 0)
                if qi < 11:
                    emit_B_gu(qi, 1, bo_q[qi][1])
                if 1 <= qi <= 11:
                    emit_C2_jp(qi - 1, cm_q[qi - 1][0], cm_q[qi - 1][1], h_q[qi - 1], 1)
                if qi < 11:
                    cg = cmp_.tile([P, 4, HCHUNK], cdt, tag="cm", name=f"cg{qi}")
                    cu = cmp_.tile([P, 4, HCHUNK], cdt, tag="cm", name=f"cu{qi}")
                    cm_q[qi] = (cg, cu)
                    emit_Cc_gu(qi, 0, bo_q[qi][0], cg)
                if 2 <= qi <= 12:
                    q = qi - 2
                    yh_q[q] = yhp.tile([P, 4, HCHUNK], cdt, tag="yh", name=f"yh{q}")
                    emit_D1_cp(q, h_q[q], yh_q[q], 0)
                if qi < 11:
                    emit_Cc_gu(qi, 1, bo_q[qi][1], cm_q[qi][1])
                if 2 <= qi <= 12:
                    q = qi - 2
                    emit_D1_cp(q, h_q[q], yh_q[q], 1)
                    h_q[q] = None
                    cm_q[q] = None
                    bo_q[q] = None

            # ---- D-combine + E interleaved per kt ----
            d_tiles = [None] * 8   # [p2pair*4 + kt] -> [P, 2, HCHUNK]

            def emit_Dc(kt, cs, zh):
                for c in cs:
                    zt = zhp.tile([P, 3, HCHUNK], cdt, tag="zh", name=f"zh{kt}{c}")
                    pz = ps.tile([P, H2], f32, tag="ta")
                    for half in range(2):
                        qgi = half
                        o0 = half * HCHUNK
                        for s, q in enumerate(QG[qgi]):
                            nc.tensor.matmul(
                                pz[32 * s:32 * s + 32, o0:o0 + HCHUNK],
                                cw_sb[:, kt, c, :], yh_q[q][:, c, :],
                                start=True, stop=True, tile_position=(0, 32 * s),
                            )
                    if c % 2 == 0:
                        nc.scalar.copy(zt[:, 0:2, :], pz)
                    else:
                        nc.vector.tensor_copy(zt[:, 0:2, :], pz)
                    pz2 = ps.tile([P, H2], f32, tag="ta")
                    for s, q in enumerate(QG[2]):
                        nc.tensor.matmul(
                            pz2[32 * s:32 * s + 32, 0:HCHUNK],
                            cw_sb[:, kt, c, :], yh_q[q][:, c, :],
                            start=True, stop=True, tile_position=(0, 32 * s),
                        )
                    if c % 2 == 0:
                        nc.vector.tensor_copy(zt[:, 2, :], pz2[:, 0:HCHUNK])
                    else:
                        nc.scalar.copy(zt[:, 2, :], pz2[:, 0:HCHUNK])
                    zh[c] = zt
                return zh

            def emit_E(kt, zh, pps):
                for pp in pps:
                    pe = ps.tile([P, H2], f32, tag="tb")
                    for half in range(2):
                        p2 = 2 * pp + half
                        o0 = half * HCHUNK
                        for qgi in range(3):
                            for c in range(4):
                                nc.tensor.matmul(
                                    pe[32 * c:32 * c + 32, o0:o0 + HCHUNK],
                                    we_sb[:, p2, kt, c, qgi, :], zh[c][:, qgi, :],
                                    start=(qgi == 0), stop=(qgi == 2),
                                    tile_position=(0, 32 * c),
                                )
                    t_ = dp.tile([P, 2, HCHUNK], cdt, tag="d")
                    if (kt + pp) % 2 == 0:
                        nc.vector.tensor_copy(t_, pe)
                    else:
                        nc.scalar.copy(t_, pe)
                    d_tiles[pp * 4 + kt] = t_

            zh_cur = [None] * 4
            emit_Dc(0, range(4), zh_cur)
            for kt in range(4):
                zh_nxt = [None] * 4
                if kt + 1 < 4:
                    emit_Dc(kt + 1, (0, 1), zh_nxt)
                emit_E(kt, zh_cur, (0,))
                if kt + 1 < 4:
                    emit_Dc(kt + 1, (2, 3), zh_nxt)
                emit_E(kt, zh_cur, (1,))
                zh_cur = zh_nxt

            # ---- F: dense iH -> output, interleaved with next chunk's A ----
            zx_nxt = [None] * 8
            fa_i = 0
            for p2 in range(4):
                for mp in range(2):
                    pf = ps.tile([P, H2], f32, tag="tb")
                    for half in range(2):
                        mt = 2 * mp + half
                        o0 = half * HCHUNK
                        for kt in range(4):
                            nc.tensor.matmul(
                                pf[:, o0:o0 + HCHUNK], ti_sb[:, kt, mt, :],
                                d_tiles[(p2 // 2) * 4 + kt][:, p2 % 2, :],
                                start=(kt == 0), stop=(kt == 3),
                            )
                    o_sb = op.tile([P, H2], f32, tag="o")
                    if (p2 + mp) % 2 == 0:
                        nc.scalar.copy(o_sb, pf)
                    else:
                        nc.vector.tensor_copy(o_sb, pf)
                    for half in range(2):
                        nc.sync.dma_start(
                            out=out[p2 * 4 + 2 * mp + half][:, ic * HCHUNK:(ic + 1) * HCHUNK],
                            in_=o_sb[:, half * HCHUNK:(half + 1) * HCHUNK],
                        )
                    if ic + 1 < N_HCH:
                        emit_A_tile(x_next, fa_i // 2, fa_i % 2, zx_nxt)
                    fa_i += 1
            zx_cur = zx_nxt

    nc.finalize()
    _built["hart3"] = nc
    return nc


def _hart3_in_maps(x, w_gate, w_up, w_down):
    quads, FR, RR, H128, T_f, T_i, CW, CW2 = _hart3_consts()
    bf = ml_dtypes.bfloat16
    U = np.zeros((4, 4, 16), np.int64)
    for kt in range(4):
        for c in range(4):
            for t in range(16):
                U[kt, c, t] = quads[16 * c + t][kt]

    tfa_pack = np.ascontiguousarray(
        T_f.reshape(4, P, 4, 4, 32).transpose(1, 0, 2, 3, 4)
    ).reshape(P, 2048).astype(bf)
    ti_pack = np.ascontiguousarray(
        T_i.reshape(4, P, 4, P).transpose(1, 0, 2, 3)
    ).reshape(P, 2048).astype(bf)

    # icw[j,c][32kt+i, a] = CW2[128kt+32c+i, 128j+32c+a]
    icw = np.zeros((4, 4, P, 32))
    for j in range(4):
        for c in range(4):
            for kt in range(4):
                rows = 128 * kt + 32 * c + np.arange(32)
                mids = 128 * j + 32 * c + np.arange(32)
                icw[j, c][32 * kt:32 * kt + 32] = CW2[np.ix_(rows, mids)]
    icw_pack = np.ascontiguousarray(icw.transpose(2, 0, 1, 3)).reshape(P, 512).astype(bf)
    isw_pack = H128[RR].astype(bf)

    sw = np.zeros((4, P, 32))
    for c in range(4):
        for idx in range(32):
            sw[c][:, idx] = H128[RR[32 * c + idx], :]
    sw_pack = np.ascontiguousarray(sw.transpose(1, 0, 2)).reshape(P, 128).astype(bf)

    cw = np.zeros((4, 4, P, 32))
    for kt in range(4):
        for c in range(4):
            rows_out = 128 * kt + 32 * c + np.arange(32)
            for j in range(4):
                mids = 128 * j + 32 * c + np.arange(32)
                cw[kt, c][32 * j:32 * j + 32] = CW[np.ix_(mids, rows_out)]
    cw_pack = np.ascontiguousarray(cw.transpose(2, 0, 1, 3)).reshape(P, 512).astype(bf)

    bg = _mix_blocks(np.asarray(w_gate, np.float64))
    bu = _mix_blocks(np.asarray(w_up, np.float64))
    bd = _mix_blocks(np.asarray(w_down, np.float64))

    # wb[32p+2t+b, q, kt, c, gu, 2t+a] = bl[q,p,U[kt,c,t],b,a]
    arr = np.stack([bg, bu], axis=0)           # [gu, q, p, unit, b, a]
    sel = arr[:, :, :, U]                      # [gu, q, p, kt, c, t, b, a]
    tmp = sel.transpose(2, 5, 6, 1, 3, 4, 0, 7)  # [p, t, b, q, kt, c, gu, a]
    wb = np.zeros((4, 16, 2, 11, 4, 4, 2, 16, 2))
    for t in range(16):
        wb[:, t, :, :, :, :, :, t, :] = tmp[:, t]
    wb_pack = wb.reshape(P, 11, 4, 4, 2, 32).reshape(P, 11264).astype(bf)

    # we[32s+2t+b, p2, kt, c, qgi, 2t+a] = bd[p2, QG[qgi][s], U[kt,c,t], b, a]
    seld = bd[:, :, U]                         # [p2, q, kt, c, t, b, a]
    we = np.zeros((4, 16, 2, 4, 4, 4, 3, 16, 2))
    for qgi, qs in enumerate(QG):
        for s, q in enumerate(qs):
            for t in range(16):
                # [p2, kt, c, b, a] -> [b, p2, kt, c, a]
                we[s, t, :, :, :, :, qgi, t, :] = seld[:, q, :, :, t].transpose(3, 0, 1, 2, 4)
    we_pack = we.reshape(P, 4, 4, 4, 3, 32).reshape(P, 6144).astype(bf)

    xf = np.asarray(x, np.float32).reshape(TOK_TOTAL, D_MODEL)
    in_maps = []
    for c0 in range(N_CORES):
        xc = xf[c0 * TOK_CORE:(c0 + 1) * TOK_CORE]
        xt = np.ascontiguousarray(
            xc.reshape(N_HCH, HCHUNK, KT, P).transpose(0, 3, 2, 1)
        ).reshape(N_HCH, P, KT * HCHUNK).astype(bf)
        in_maps.append({
            "xT": xt, "tfa": tfa_pack, "tid": ti_pack, "wbd": wb_pack,
            "wed": we_pack, "icwd": icw_pack, "iswd": isw_pack,
            "swd": sw_pack, "cwd": cw_pack,
        })
    return in_maps


def _build_hart2():
    """Col-tiled Hartley kernel: the 2x2-block-diagonal mix tiles are packed
    into 32-partition strips so 4 concurrent M=32 col-tiled matmuls replace 4
    serial 128x128 ones (mix stages run ~4x denser on the PE array).

    Per 512-token chunk (PE slots of ~216ns):
      A  x->packed hx      256 MMs (4x col-conc)  ~64 slots
      B  mix gate/up       352 MMs (4x col-conc)  ~88
      C  iH on gate/up     352 MMs dense           352
      D  h->packed Hh      704 MMs (4x col-conc) ~176
      E  mix down          192 MMs (4x col-conc)  ~48
      F  iH -> output       64 MMs dense            64
    total ~792 slots/chunk vs 1184 for mode "hart".
    """
    if "hart2" in _built:
        return _built["hart2"]
    cdt = mybir.dt.bfloat16
    f32 = mybir.dt.float32
    nc = bacc.Bacc("TRN2", debug=False, num_devices=N_CORES)

    xT = nc.dram_tensor("xT", [N_HCH, P, KT * HCHUNK], cdt, kind="ExternalInput").ap()
    tfa = nc.dram_tensor("tfa", [P, 4 * 4 * 4 * 32], cdt, kind="ExternalInput").ap()
    tid = nc.dram_tensor("tid", [P, 16 * P], cdt, kind="ExternalInput").ap()
    wb = nc.dram_tensor("wb", [P, 11 * 4 * 2 * 4 * 32], cdt, kind="ExternalInput").ap()
    we = nc.dram_tensor("we", [P, 4 * 4 * 4 * 3 * 32], cdt, kind="ExternalInput").ap()
    out = nc.dram_tensor("outT", [KT, P, TOK_CORE], f32, kind="ExternalOutput").ap()

    with tile.TileContext(nc) as tc, ExitStack() as ctx:
        wpool = ctx.enter_context(tc.tile_pool(name="wpool", bufs=1))
        xp = ctx.enter_context(tc.tile_pool(name="xp", bufs=8))
        zxp = ctx.enter_context(tc.tile_pool(name="zxp", bufs=8))
        gup = ctx.enter_context(tc.tile_pool(name="gup", bufs=4))
        sp = ctx.enter_context(tc.tile_pool(name="sp", bufs=2))
        hp = ctx.enter_context(tc.tile_pool(name="hp", bufs=11))
        zhp = ctx.enter_context(tc.tile_pool(name="zhp", bufs=24))
        dp = ctx.enter_context(tc.tile_pool(name="dp", bufs=17))
        op = ctx.enter_context(tc.tile_pool(name="op", bufs=2))
        ps = ctx.enter_context(tc.tile_pool(name="ps", bufs=2, space="PSUM"))

        tfa_sb = wpool.tile([P, 4, 4, 4, 32], cdt, tag="tfa")
        ti_sb = wpool.tile([P, 4, 4, P], cdt, tag="ti")
        wb_sb = wpool.tile([P, 11, 4, 2, 4, 32], cdt, tag="wb")
        we_sb = wpool.tile([P, 4, 4, 4, 3, 32], cdt, tag="we")

        # warm the PE while the first DMAs are in flight
        wz = sp.tile([P, HCHUNK], cdt, tag="wz", bufs=1)
        nc.vector.memset(wz, 0.0)
        wps = ps.tile([P, HCHUNK], f32, tag="td")
        for wi in range(40):
            nc.tensor.matmul(wps[:, :HCHUNK], wz[:, :P], wz, start=(wi == 0), stop=(wi == 39))
        wdrain = sp.tile([P, 4], f32, tag="wdrain", bufs=1)
        nc.vector.tensor_copy(wdrain, wps[:, :4])

        def load_x(ic):
            ts_ = []
            for p_ in range(4):
                t = xp.tile([P, 4, HCHUNK], cdt, tag="x")
                xr = xT[ic][:, p_ * 4 * HCHUNK:(p_ + 1) * 4 * HCHUNK].rearrange(
                    "p (kt t) -> p kt t", kt=4
                )
                for kin in range(4):
                    nc.sync.dma_start(out=t[:, kin, :], in_=xr[:, kin, :])
                ts_.append(t)
            return ts_

        x_next = load_x(0)
        nc.sync.dma_start(out=tfa_sb, in_=tfa.rearrange("p (a b c m) -> p a b c m", a=4, b=4, c=4))
        nc.sync.dma_start(out=ti_sb, in_=tid.rearrange("p (a b m) -> p a b m", a=4, b=4))
        wb_r = wb.rearrange("p (q a g c m) -> p q a g c m", q=11, a=4, g=2, c=4)
        for q2 in range(11):
            nc.sync.dma_start(out=wb_sb[:, q2], in_=wb_r[:, q2])
        nc.sync.dma_start(out=we_sb, in_=we.rearrange("p (a b c d m) -> p a b c d m", a=4, b=4, c=4, d=3))

        for ic in range(N_HCH):
            x_sb = x_next

            # ---- A: packed hx tiles Zx[kt,g] = [16 units(g) x 4 p-strips, T]
            zx = [None] * 16
            for kt in range(4):
                for g in range(4):
                    pa = ps.tile([P, HCHUNK], f32, tag="ta")
                    for kin in range(4):
                        for p_ in range(4):
                            nc.tensor.matmul(
                                pa[32 * p_:32 * p_ + 32, :],
                                tfa_sb[:, kin, kt, g, :], x_sb[p_][:, kin, :],
                                start=(kin == 0), stop=(kin == 3),
                                tile_position=(0, 32 * p_),
                            )
                    t_ = zxp.tile([P, HCHUNK], cdt, tag="zx")
                    nc.scalar.copy(t_, pa)
                    zx[kt * 4 + g] = t_

            if ic + 1 < N_HCH:
                x_next = load_x(ic + 1)

            # ---- B (col-tiled mix) + C (dense iH) interleaved per q ----
            def emit_B(q):
                gq = gup.tile([P, 4, HCHUNK], cdt, tag="gu")
                uq = gup.tile([P, 4, HCHUNK], cdt, tag="gu")
                for kt in range(4):
                    for gu, dst in ((0, gq), (1, uq)):
                        pb = ps.tile([P, HCHUNK], f32, tag="tb")
                        for g in range(4):
                            nc.tensor.matmul(
                                pb[32 * g:32 * g + 32, :],
                                wb_sb[:, q, kt, gu, g, :], zx[kt * 4 + g],
                                start=True, stop=True, tile_position=(0, 32 * g),
                            )
                        nc.vector.tensor_copy(dst[:, kt, :], pb)
                return gq, uq

            def emit_C(q, gq, uq, hq):
                for mt in range(4):
                    pg = ps.tile([P, HCHUNK], f32, tag="tc")
                    for kt in range(4):
                        nc.tensor.matmul(pg, ti_sb[:, kt, mt, :], gq[:, kt, :],
                                         start=(kt == 0), stop=(kt == 3))
                    sg = sp.tile([P, HCHUNK], f32, tag="sg")
                    nc.scalar.activation(sg, pg, mybir.ActivationFunctionType.Silu)
                    pu = ps.tile([P, HCHUNK], f32, tag="tc")
                    for kt in range(4):
                        nc.tensor.matmul(pu, ti_sb[:, kt, mt, :], uq[:, kt, :],
                                         start=(kt == 0), stop=(kt == 3))
                    nc.vector.tensor_mul(hq[:, mt, :], sg, pu)

            h = [hp.tile([P, 4, HCHUNK], cdt, tag="h", name=f"h{qi}") for qi in range(11)]
            cur = emit_B(0)
            for q in range(11):
                nxt = emit_B(q + 1) if q + 1 < 11 else None
                emit_C(q, cur[0], cur[1], h[q])
                cur = nxt

            # ---- D (h -> packed Hh) + E (mix down) interleaved per kt ----
            d_tiles = [None] * 16

            def emit_D(kt):
                zh = [None] * 12
                for g in range(4):
                    for qgi, qs in enumerate(QG):
                        pa = ps.tile([P, HCHUNK], f32, tag="ta")
                        for kin in range(4):
                            for s, q in enumerate(qs):
                                nc.tensor.matmul(
                                    pa[32 * s:32 * s + 32, :],
                                    tfa_sb[:, kin, kt, g, :], h[q][:, kin, :],
                                    start=(kin == 0), stop=(kin == 3),
                                    tile_position=(0, 32 * s),
                                )
                        t_ = zhp.tile([P, HCHUNK], cdt, tag="zh")
                        nc.scalar.copy(t_, pa)
                        zh[g * 3 + qgi] = t_
                return zh

            def emit_E(kt, zh):
                for p2 in range(4):
                    pe = ps.tile([P, HCHUNK], f32, tag="tb")
                    for qgi in range(3):
                        for g in range(4):
                            nc.tensor.matmul(
                                pe[32 * g:32 * g + 32, :],
                                we_sb[:, p2, kt, g, qgi, :], zh[g * 3 + qgi],
                                start=(qgi == 0), stop=(qgi == 2),
                                tile_position=(0, 32 * g),
                            )
                    t_ = dp.tile([P, HCHUNK], cdt, tag="d")
                    nc.vector.tensor_copy(t_, pe)
                    d_tiles[p2 * 4 + kt] = t_

            zh_cur = emit_D(0)
            for kt in range(4):
                zh_nxt = emit_D(kt + 1) if kt + 1 < 4 else None
                emit_E(kt, zh_cur)
                zh_cur = zh_nxt

            # ---- F: iH -> real output ----
            for p2 in range(4):
                for mt in range(4):
                    pf = ps.tile([P, HCHUNK], f32, tag="td")
                    for kt in range(4):
                        nc.tensor.matmul(pf, ti_sb[:, kt, mt, :], d_tiles[p2 * 4 + kt],
                                         start=(kt == 0), stop=(kt == 3))
                    o_sb = op.tile([P, HCHUNK], f32, tag="o")
                    nc.scalar.copy(o_sb, pf)
                    nc.sync.dma_start(
                        out=out[p2 * 4 + mt][:, ic * HCHUNK:(ic + 1) * HCHUNK], in_=o_sb
                    )

    nc.finalize()
    _built["hart2"] = nc
    return nc


def _hart2_in_maps(x, w_gate, w_up, w_down):
    T_f, T_i, R = _hartley_mats()
    bf = ml_dtypes.bfloat16

    tfa_pack = np.ascontiguousarray(
        T_f.reshape(4, P, 4, 4, 32).transpose(1, 0, 2, 3, 4)
    ).reshape(P, 4 * 4 * 4 * 32).astype(bf)
    ti_pack = np.ascontiguousarray(
        T_i.reshape(4, P, 4, P).transpose(1, 0, 2, 3)
    ).reshape(P, 16 * P).astype(bf)

    tg = _mix_tiles(np.asarray(w_gate, np.float32), R)   # [11,4,4,128,128]
    tu = _mix_tiles(np.asarray(w_up, np.float32), R)
    td = _mix_tiles(np.asarray(w_down, np.float32), R)   # [4,11,4,128,128]

    def diag_strips(t):  # [q,p,kt,128,128] -> [q,p,kt,g,32,32]
        return np.stack([t[..., 32 * g:32 * g + 32, 32 * g:32 * g + 32]
                         for g in range(4)], axis=3)

    gb = diag_strips(tg)  # [11,4(p),4(kt),4(g),32(r),32(c)]
    ub = diag_strips(tu)
    # wb[32p+r, q, kt, gu, g, c]
    wb_pack = np.stack([
        gb.transpose(1, 4, 0, 2, 3, 5).reshape(P, 11, 4, 4, 32),
        ub.transpose(1, 4, 0, 2, 3, 5).reshape(P, 11, 4, 4, 32),
    ], axis=3).reshape(P, 11 * 4 * 2 * 4 * 32).astype(bf)

    db = diag_strips(td)  # [4(p2),11(q),4(kt),4(g),32(r),32(c)]
    # we[32s+r, p2, kt, g, qgi, c]
    we_pack = np.zeros((4, 32, 4, 4, 4, 3, 32), np.float32)  # [s,r,p2,kt,g,qgi,c]
    for qgi, qs in enumerate(QG):
        for s, q in enumerate(qs):
            we_pack[s, :, :, :, :, qgi, :] = db[:, q].transpose(3, 0, 1, 2, 4)
    we_pack = we_pack.reshape(P, 4, 4, 4, 3, 32).reshape(P, -1).astype(bf)

    xf = np.asarray(x, np.float32).reshape(TOK_TOTAL, D_MODEL)
    in_maps = []
    for c in range(N_CORES):
        xc = xf[c * TOK_CORE:(c + 1) * TOK_CORE]
        xt = np.ascontiguousarray(
            xc.reshape(N_HCH, HCHUNK, KT, P).transpose(0, 3, 2, 1)
        ).reshape(N_HCH, P, KT * HCHUNK).astype(bf)
        in_maps.append({
            "xT": xt, "tfa": tfa_pack, "tid": ti_pack,
            "wb": wb_pack, "we": we_pack,
        })
    return in_maps


def _hart_in_maps(x, w_gate, w_up, w_down):
    T_f, T_i, R = _hartley_mats()
    bf = ml_dtypes.bfloat16

    tf_pack = np.ascontiguousarray(
        T_f.reshape(4, P, 4, P).transpose(1, 0, 2, 3)
    ).reshape(P, 16 * P).astype(bf)
    ti_pack = np.ascontiguousarray(
        T_i.reshape(4, P, 4, P).transpose(1, 0, 2, 3)
    ).reshape(P, 16 * P).astype(bf)

    tg = _mix_tiles(np.asarray(w_gate, np.float32), R)   # [11,4,4,128,128]
    tu = _mix_tiles(np.asarray(w_up, np.float32), R)
    td = _mix_tiles(np.asarray(w_down, np.float32), R)   # [4,11,4,128,128]
    mg_pack = np.ascontiguousarray(tg.transpose(0, 2, 3, 1, 4)).reshape(44, P, 4 * P)
    mu_pack = np.ascontiguousarray(tu.transpose(0, 2, 3, 1, 4)).reshape(44, P, 4 * P)
    mgu_pack = np.concatenate([mg_pack[:, :, None], mu_pack[:, :, None]], axis=2)
    mgu_pack = mgu_pack.reshape(44, P, 8 * P).astype(bf)
    md_pack = np.ascontiguousarray(td.transpose(0, 2, 3, 1, 4)).reshape(16, P, 11 * P).astype(bf)

    xf = np.asarray(x, np.float32).reshape(TOK_TOTAL, D_MODEL)
    in_maps = []
    for c in range(N_CORES):
        xc = xf[c * TOK_CORE:(c + 1) * TOK_CORE]
        xt = np.ascontiguousarray(
            xc.reshape(N_HCH, HCHUNK, KT, P).transpose(0, 3, 2, 1)
        ).reshape(N_HCH, P, KT * HCHUNK).astype(bf)
        in_maps.append({
            "xT": xt, "tf": tf_pack, "ti": ti_pack,
            "mgu": mgu_pack, "md": md_pack,
        })
    return in_maps


def _materialize(w):
    """(q, p, b) circulant generators -> dense [p*b, q*b] (in-dim, out-dim)."""
    q, p, b = w.shape
    i = np.arange(b)
    idx = (i[None, :] - i[:, None]) % b          # [j, i]
    return w[:, :, idx].transpose(1, 2, 0, 3).reshape(p * b, q * b)


def kernel(x, w_gate, w_up, w_down):
    mode = MODE
    if mode == "hart3":
        nc = _build_hart3()
        in_maps = _hart3_in_maps(x, w_gate, w_up, w_down)
        return _run(nc, in_maps)
    if mode == "hart2":
        nc = _build_hart2()
        in_maps = _hart2_in_maps(x, w_gate, w_up, w_down)
        return _run(nc, in_maps)
    if mode == "hart":
        nc = _build_hart()
        in_maps = _hart_in_maps(x, w_gate, w_up, w_down)
        return _run(nc, in_maps)
    cdt, npdt, pass_t, mm_n, _ = _MODE_CFG[mode]
    n_pass = TOK_CORE // pass_t

    nc = _build(mode)

    Wg = _materialize(np.asarray(w_gate, np.float32))   # [2048, 5632]
    Wu = _materialize(np.asarray(w_up, np.float32))     # [2048, 5632]
    Wd = _materialize(np.asarray(w_down, np.float32))   # [5632, 2048]

    # wgu packed: [MT, P, 2, KT, P]; per-partition rows contiguous
    wgu = np.empty((MT, P, 2, KT, P), np.float32)
    wg4 = Wg.reshape(KT, P, MT, P)   # [k, kp, m, mp]
    wu4 = Wu.reshape(KT, P, MT, P)
    wgu[:, :, 0] = wg4.transpose(2, 1, 0, 3)  # [m, kp, k, mp]
    wgu[:, :, 1] = wu4.transpose(2, 1, 0, 3)
    wgu = wgu.reshape(MT, P, 2 * KT * P).astype(npdt)

    wd4 = Wd.reshape(MT, P, KT, P)   # [k2, kp, m2, mp]
    wdp = np.ascontiguousarray(wd4.transpose(2, 1, 0, 3)).reshape(KT, P, MT * P).astype(npdt)

    xf = np.asarray(x, np.float32).reshape(TOK_TOTAL, D_MODEL)
    in_maps = []
    for c in range(N_CORES):
        xc = xf[c * TOK_CORE:(c + 1) * TOK_CORE]          # [2048 tok, 2048 d]
        # -> [n_pass, P, KT, pass_t]: xT[pass, kp, k, t] = xc[pass*pt+t, k*P+kp]
        xt = np.ascontiguousarray(
            xc.reshape(n_pass, pass_t, KT, P).transpose(0, 3, 2, 1)
        ).reshape(n_pass, P, KT * pass_t).astype(npdt)
        in_maps.append({"xT": xt, "wgu": wgu, "wd": wdp})

    return _run(nc, in_maps)


def _run(nc, in_maps):
    trace = bool(os.environ.get("BASS_PROFILE"))
    try:
        res = run_bass_kernel_spmd(nc, in_maps, core_ids=list(range(N_CORES)), trace=trace)
    except Exception:
        # transient device wedge (e.g. NRT_EXEC_UNIT_UNRECOVERABLE) - retry once
        import time as _t
        _t.sleep(5)
        res = run_bass_kernel_spmd(nc, in_maps, core_ids=list(range(N_CORES)), trace=trace)
    global last_results
    last_results = res

    out = np.empty((TOK_TOTAL, D_MODEL), np.float32)
    for c in range(N_CORES):
        o = res.results[c]["outT"]                         # [KT, P, TOK_CORE]
        out[c * TOK_CORE:(c + 1) * TOK_CORE] = o.reshape(D_MODEL, TOK_CORE).T
    return out.reshape(4, 4096, D_MODEL)

